# revision 4
# baseline (speedup 1.0000x reference)
"""Trainium2 Bass kernel for sliding-window causal attention block.

Reference computation (B=2, S=2048, D=1024, H=16, hd=64, WINDOW=256):
    c = x @ w_attn + b_attn ; q,k,v = split(c)
    present = stack([k, v]) as [B,2,H,S,hd]
    att = softmax(mask(q k^T / sqrt(hd))) @ v
    out = att @ w_proj + b_proj

Sharding: 8 cores = 2 batches x 4 head-groups (4 heads each).
Per core: QKV projection for its 256 q/k/v features (column-sharded
w_attn), attention for its 4 heads, and a partial out-projection
(row-sharded w_proj). Host sums the 4 partials per batch and adds
b_proj exactly.

Layout strategy on-core:
  x[b] is PE-transposed once to xT [D, S] (fp32, exact).
  Q^T, K^T produced feature-major [feat, S] (f32r), V natural [S, feat].
  Scores are computed directly transposed: S^T[kpos, q] tile per
  128-kpos chunk covering its 256 valid q columns, so softmax needs no
  P transposes. Because WINDOW=256=2 tiles, the mask is just a causal
  triangle on the left 128 q columns of each chunk.
  exp on ACT -> P^T fp16; AV matmul uses V|ones fp16 (ones column
  yields the softmax denominator for free); per-q-tile normalization:
  DVE reciprocal + GPSIMD partition_broadcast + DVE multiply -> O^T
  f32r; out-projection back to natural [S, D] layout.
"""

import sys

sys.path.insert(0, "/opt/trn_rl_repo")

import numpy as np

import concourse.bass as bass  # noqa: F401  (bass must import before bacc)
import concourse.mybir as mybir
from concourse import bacc
from concourse.tile import TileContext
from concourse.bass_utils import run_bass_kernel_spmd

F32 = mybir.dt.float32
F32R = mybir.dt.float32r
F16 = mybir.dt.float16

B, S, D = 2, 2048, 1024
N_HEAD = 16
HD = 64
WINDOW = 256
NCORES = 8
HPC = N_HEAD // 4  # heads per core = 4
FPC = HPC * HD  # features per core = 256
NQT = S // 128  # 16 q/kpos tiles
NDC = D // 128  # 8 contraction chunks
SCALE = 1.0 / np.sqrt(HD)

_CACHE = {}


def _build_program():
    nc = bacc.Bacc("TRN2", target_bir_lowering=False, debug=False,
                   num_devices=NCORES)

    # ---- DRAM I/O ----
    xb = nc.dram_tensor("xb", [S, D], F32, kind="ExternalInput")
    wq = nc.dram_tensor("wq", [128, NDC * FPC], F32R, kind="ExternalInput")
    wk = nc.dram_tensor("wk", [128, NDC * FPC], F32R, kind="ExternalInput")
    wv = nc.dram_tensor("wv", [128, NDC * FPC], F32R, kind="ExternalInput")
    wp = nc.dram_tensor("wp", [128, 2 * D], F32R, kind="ExternalInput")
    bq = nc.dram_tensor("bq", [128, 2], F32, kind="ExternalInput")
    bk = nc.dram_tensor("bk", [128, 2], F32, kind="ExternalInput")
    bv = nc.dram_tensor("bv", [1, FPC], F32R, kind="ExternalInput")
    ident = nc.dram_tensor("ident", [128, 128], F32, kind="ExternalInput")
    ident_r = nc.dram_tensor("ident_r", [128, 128], F32R, kind="ExternalInput")
    tri = nc.dram_tensor("tri", [128, 128], F16, kind="ExternalInput")
    atri = nc.dram_tensor("atri", [128, 128], F16, kind="ExternalInput")
    onesr = nc.dram_tensor("onesr", [1, 128], F32R, kind="ExternalInput")

    outp = nc.dram_tensor("outp", [S, D], F32, kind="ExternalOutput")
    pk = nc.dram_tensor("pk", [HPC, S, HD], F32, kind="ExternalOutput")
    pv = nc.dram_tensor("pv", [HPC, S, HD], F32, kind="ExternalOutput")

    Exp = mybir.ActivationFunctionType.Exp
    Ident = mybir.ActivationFunctionType.Identity
    Copy = mybir.ActivationFunctionType.Copy

    with TileContext(nc) as tc:
        with tc.tile_pool(name="const", bufs=1) as cpool, \
             tc.tile_pool(name="qkv", bufs=1) as qkv, \
             tc.tile_pool(name="ot", bufs=1) as otp:

            # ---- constants / weights ----
            wq_sb = cpool.tile([128, NDC * FPC], F32R, tag="wq")
            nc.sync.dma_start(out=wq_sb[:], in_=wq[:])
            wk_sb = cpool.tile([128, NDC * FPC], F32R, tag="wk")
            nc.sync.dma_start(out=wk_sb[:], in_=wk[:])
            wv_sb = cpool.tile([128, NDC * FPC], F32R, tag="wv")
            nc.sync.dma_start(out=wv_sb[:], in_=wv[:])
            wp_sb = cpool.tile([128, 2 * D], F32R, tag="wp")
            nc.sync.dma_start(out=wp_sb[:], in_=wp[:])
            bq_sb = cpool.tile([128, 2], F32, tag="bq")
            nc.sync.dma_start(out=bq_sb[:], in_=bq[:])
            bk_sb = cpool.tile([128, 2], F32, tag="bk")
            nc.sync.dma_start(out=bk_sb[:], in_=bk[:])
            bv_sb = cpool.tile([1, FPC], F32R, tag="bv")
            nc.sync.dma_start(out=bv_sb[:], in_=bv[:])
            id_sb = cpool.tile([128, 128], F32, tag="id")
            nc.sync.dma_start(out=id_sb[:], in_=ident[:])
            idr_sb = cpool.tile([128, 128], F32R, tag="idr")
            nc.sync.dma_start(out=idr_sb[:], in_=ident_r[:])
            tri_sb = cpool.tile([128, 128], F16, tag="tri")
            nc.sync.dma_start(out=tri_sb[:], in_=tri[:])
            atri_sb = cpool.tile([128, 128], F16, tag="atri")
            nc.sync.dma_start(out=atri_sb[:], in_=atri[:])
            on_sb = cpool.tile([1, 128], F32R, tag="on")
            nc.sync.dma_start(out=on_sb[:], in_=onesr[:])

            # ---- persistent activations ----
            # Q^T/K^T: 2 feat-tiles x [128, S] (partitions=features)
            qt_sb = [qkv.tile([128, S], F32R, tag=f"qt{i}", name=f"qt{i}")
                     for i in range(2)]
            kt_sb = [qkv.tile([128, S], F32R, tag=f"kt{i}", name=f"kt{i}")
                     for i in range(2)]
            # V natural [S-tiles along free]: [128, 16*256]
            v_sb = qkv.tile([128, NQT * FPC], F32, tag="v")
            # V|ones fp16 per head: head hl block is [128, NQT*65]
            vb_sb = qkv.tile([128, HPC * NQT * 65], F16, tag="vb")
            # O^T (normalized, f32r): 2 feat-tiles x [128, S]
            ot_sb = [otp.tile([128, S], F32R, tag=f"ot{i}", name=f"ot{i}")
                     for i in range(2)]

            nc.gpsimd.memset(vb_sb[:], 1.0)

            # ---- stage 1: load x tiles, transpose to xT ----
            with tc.tile_pool(name="xt", bufs=1) as xtp, \
                 tc.tile_pool(name="xload", bufs=3) as xlp, \
                 tc.tile_pool(name="tp_ps", bufs=2, space="PSUM") as tpps, \
                 tc.tile_pool(name="qk_ps", bufs=4, space="PSUM") as qkps, \
                 tc.tile_pool(name="vp_ps", bufs=2, space="PSUM") as vpps:

                xt_sb = xtp.tile([128, NDC * S], F32R, tag="xt")
                for st in range(NQT):
                    x_t = xlp.tile([128, D], F32, tag="x")
                    nc.sync.dma_start(out=x_t[:], in_=xb[st * 128:(st + 1) * 128, :])
                    for dc in range(NDC):
                        ps = tpps.tile([128, 128], F32, tag="tp")
                        nc.tensor.transpose(ps[:], x_t[:, dc * 128:(dc + 1) * 128],
                                            id_sb[:])
                        nc.scalar.copy(
                            xt_sb[:, dc * S + st * 128: dc * S + st * 128 + 128],
                            ps[:])

                # ---- stage 2: QKV projection ----
                # Q^T and K^T (feature-major)
                for w_sb, b_sb, dst in ((wq_sb, bq_sb, qt_sb),
                                        (wk_sb, bk_sb, kt_sb)):
                    for ft in range(2):
                        pss = [qkps.tile([128, 512], F32, tag="qk",
                                         name=f"qkps{ft}_{i}")
                               for i in range(4)]
                        for dc in range(NDC):
                            lhsT = w_sb[:, dc * FPC + ft * 128:
                                        dc * FPC + ft * 128 + 128]
                            for sc in range(4):
                                nc.tensor.matmul(
                                    pss[sc][:], lhsT,
                                    xt_sb[:, dc * S + sc * 512:
                                          dc * S + sc * 512 + 512],
                                    start=(dc == 0), stop=(dc == NDC - 1))
                        for sc in range(4):
                            nc.scalar.activation(
                                dst[ft][:, sc * 512:(sc + 1) * 512],
                                pss[sc][:], Ident,
                                bias=b_sb[:, ft:ft + 1])

                # V natural + fused bias; also fp16 V|ones blocks
                for st in range(NQT):
                    vp = vpps.tile([128, FPC], F32, tag="vp")
                    for dc in range(NDC):
                        nc.tensor.matmul(
                            vp[:],
                            xt_sb[:, dc * S + st * 128: dc * S + st * 128 + 128],
                            wv_sb[:, dc * FPC:(dc + 1) * FPC],
                            start=(dc == 0), stop=False)
                    nc.tensor.matmul(vp[:], on_sb[:], bv_sb[:],
                                     start=False, stop=True)
                    nc.scalar.copy(v_sb[:, st * FPC:(st + 1) * FPC], vp[:])
                    # strided fp16 copy into the 4 per-head V|ones blocks
                    src = vp[:].rearrange("p (hl c) -> p hl c", hl=HPC)
                    dst3 = vb_sb[:].rearrange("p (hl t) -> p hl t", hl=HPC)[
                        :, :, st * 65: st * 65 + HD]
                    nc.vector.tensor_copy(dst3, src)

            # ---- stage 3: K natural for `present` output ----
            with tc.tile_pool(name="kn", bufs=3) as knp, \
                 tc.tile_pool(name="kn_ps", bufs=3, space="PSUM") as knps:
                for ft in range(2):
                    for st in range(NQT):
                        kp = knps.tile([128, 128], F32R, tag="knp")
                        nc.tensor.transpose(
                            kp[:], kt_sb[ft][:, st * 128:(st + 1) * 128],
                            idr_sb[:])
                        kn = knp.tile([128, 128], F32, tag="kn")
                        nc.scalar.copy(kn[:], kp[:].bitcast(F32))
                        dst = pk[2 * ft: 2 * ft + 2,
                                 st * 128:(st + 1) * 128, :]
                        # dram view [2, 128, 64] <- sbuf [128p, (2,64)]
                        nc.sync.dma_start(
                            out=dst.rearrange("h s d -> s h d"),
                            in_=kn[:].rearrange("p (h d) -> p h d", h=2))

                # present V: one DMA per S-tile
                for st in range(NQT):
                    nc.sync.dma_start(
                        out=pv[:, st * 128:(st + 1) * 128, :].rearrange(
                            "h s d -> s h d"),
                        in_=v_sb[:, st * FPC:(st + 1) * FPC].rearrange(
                            "p (h d) -> p h d", h=HPC))

            # ---- stage 4: attention per head ----
            with tc.tile_pool(name="pt", bufs=4) as ptp, \
                 tc.tile_pool(name="rs", bufs=4) as rsp, \
                 tc.tile_pool(name="st_ps", bufs=3, space="PSUM") as stps, \
                 tc.tile_pool(name="o_ps", bufs=5, space="PSUM") as ops:
                for hl in range(HPC):
                    ft, po = hl // 2, (hl % 2) * 64
                    kth = kt_sb[ft]
                    qth = qt_sb[ft]
                    oth = ot_sb[ft]
                    vbh = vb_sb[:, hl * NQT * 65:(hl + 1) * NQT * 65]
                    # rolling window of 3 accumulating O^T psums:
                    # q-tile t gets kpos chunks t-2 (anti-tri), t-1 (full),
                    # t (causal tri)
                    ops_live = [None, None]  # [opsum for c+1, opsum for c+2]
                    for c in range(NQT):
                        qw = min(384, S - c * 128)
                        sp = stps.tile([128, 384], F32, tag="sp")
                        nc.tensor.matmul(
                            sp[:, :qw],
                            kth[po:po + 64, c * 128:(c + 1) * 128],
                            qth[po:po + 64, c * 128:c * 128 + qw],
                            start=True, stop=True)
                        pt = ptp.tile([128, 384], F16, tag="pt")
                        nc.scalar.activation(pt[:, :qw], sp[:, :qw], Exp)
                        # causal triangle on left 128 q cols
                        nc.vector.tensor_mul(pt[:, 0:128], pt[:, 0:128],
                                             tri_sb[:])
                        # anti-causal triangle on right 128 q cols
                        if qw > 256:
                            nc.vector.tensor_mul(
                                pt[:, 256:256 + (qw - 256)],
                                pt[:, 256:256 + (qw - 256)],
                                atri_sb[:, 0:qw - 256])
                        lhsT = vbh[:, c * 65:(c + 1) * 65]
                        # finish q-tile c
                        o_cur = ops_live[0]
                        if o_cur is None:
                            o_cur = ops.tile([65, 128], F32, tag="o",
                                             name=f"o{hl}_{c}")
                        nc.tensor.matmul(o_cur[:], lhsT, pt[:, 0:128],
                                         start=(c == 0), stop=True)
                        # middle third -> q-tile c+1
                        if c + 1 < NQT:
                            o1 = ops_live[1]
                            if o1 is None:
                                o1 = ops.tile([65, 128], F32, tag="o",
                                              name=f"o{hl}_{c}m")
                            nc.tensor.matmul(o1[:], lhsT, pt[:, 128:256],
                                             start=(c == 0), stop=False)
                        else:
                            o1 = None
                        # right third -> q-tile c+2
                        if c + 2 < NQT and qw > 256:
                            o2 = ops.tile([65, 128], F32, tag="o",
                                          name=f"o{hl}_{c}r")
                            nc.tensor.matmul(o2[:], lhsT, pt[:, 256:384],
                                             start=True, stop=False)
                        else:
                            o2 = None
                        ops_live = [o1, o2]
                        # normalize q-tile c -> O^T (f32r)
                        rec = rsp.tile([1, 128], F32, tag="rec")
                        nc.vector.reciprocal(rec[:], o_cur[64:65, :])
                        rb = rsp.tile([64, 128], F32, tag="rb")
                        nc.gpsimd.partition_broadcast(rb[:], rec[:])
                        nc.vector.tensor_mul(
                            oth[po:po + 64, c * 128:(c + 1) * 128],
                            o_cur[0:64, :], rb[:])

            # ---- stage 5: out-projection (partial, natural layout) ----
            with tc.tile_pool(name="osb", bufs=3) as osbp, \
                 tc.tile_pool(name="op_ps", bufs=4, space="PSUM") as opps:
                for st in range(NQT):
                    o_t = osbp.tile([128, D], F32, tag="osb")
                    for half in range(2):
                        op = opps.tile([128, 512], F32, tag="op")
                        nc.tensor.matmul(
                            op[:], ot_sb[0][:, st * 128:(st + 1) * 128],
                            wp_sb[:, half * 512: half * 512 + 512],
                            start=True, stop=False)
                        nc.tensor.matmul(
                            op[:], ot_sb[1][:, st * 128:(st + 1) * 128],
                            wp_sb[:, D + half * 512: D + half * 512 + 512],
                            start=False, stop=True)
                        nc.scalar.copy(o_t[:, half * 512:(half + 1) * 512],
                                       op[:])
                    nc.sync.dma_start(
                        out=outp[st * 128:(st + 1) * 128, :], in_=o_t[:])

    nc.compile()
    return nc


def _prep_in_maps(x, w_attn, b_attn, w_proj):
    """Per-core input dicts (host-side sharding + layout prep)."""
    x = np.ascontiguousarray(np.asarray(x, dtype=np.float32))
    w_attn = np.asarray(w_attn, dtype=np.float32)
    b_attn = np.asarray(b_attn, dtype=np.float32)
    w_proj = np.asarray(w_proj, dtype=np.float32)

    ident = np.eye(128, dtype=np.float32)
    tri = (np.arange(128)[None, :] >= np.arange(128)[:, None]).astype(
        np.float16)
    atri = (np.arange(128)[None, :] < np.arange(128)[:, None]).astype(
        np.float16)
    onesr = np.ones((1, 128), dtype=np.float32)

    def chunk_w(w_cols):  # [D, FPC] -> [128, NDC*FPC]
        return np.ascontiguousarray(
            w_cols.reshape(NDC, 128, FPC).transpose(1, 0, 2).reshape(
                128, NDC * FPC))

    in_maps = []
    for core in range(NCORES):
        b, hg = core // 4, core % 4
        cols = slice(hg * FPC, (hg + 1) * FPC)
        kcols = slice(D + hg * FPC, D + (hg + 1) * FPC)
        vcols = slice(2 * D + hg * FPC, 2 * D + (hg + 1) * FPC)
        rows = slice(hg * FPC, (hg + 1) * FPC)
        in_maps.append({
            "xb": x[b],
            "wq": chunk_w(w_attn[:, cols] * np.float32(SCALE)),
            "wk": chunk_w(w_attn[:, kcols]),
            "wv": chunk_w(w_attn[:, vcols]),
            "wp": np.ascontiguousarray(
                w_proj[rows, :].reshape(2, 128, D).transpose(1, 0, 2).reshape(
                    128, 2 * D)),
            "bq": np.ascontiguousarray(
                (b_attn[cols] * np.float32(SCALE)).reshape(2, 128).T),
            "bk": np.ascontiguousarray(b_attn[kcols].reshape(2, 128).T),
            "bv": b_attn[vcols].reshape(1, FPC).copy(),
            "ident": ident,
            "ident_r": ident,
            "tri": tri,
            "atri": atri,
            "onesr": onesr,
        })
    return in_maps


def kernel(x, w_attn, b_attn, w_proj, b_proj):
    if "nc" not in _CACHE:
        _CACHE["nc"] = _build_program()
    nc = _CACHE["nc"]

    in_maps = _prep_in_maps(x, w_attn, b_attn, w_proj)
    res = run_bass_kernel_spmd(nc, in_maps, core_ids=list(range(NCORES)))

    b_proj = np.asarray(b_proj, dtype=np.float32)
    out = np.zeros((B, S, D), dtype=np.float32)
    present = np.zeros((B, 2, N_HEAD, S, HD), dtype=np.float32)
    for core in range(NCORES):
        b, hg = core // 4, core % 4
        r = res.results[core]
        out[b] += r["outp"]
        present[b, 0, hg * HPC:(hg + 1) * HPC] = r["pk"]
        present[b, 1, hg * HPC:(hg + 1) * HPC] = r["pv"]
    out += b_proj
    return out, present


# revision 5
# speedup vs baseline: 1.0860x; 1.0860x over previous
"""Trainium2 Bass kernel for sliding-window causal attention block.

Reference computation (B=2, S=2048, D=1024, H=16, hd=64, WINDOW=256):
    c = x @ w_attn + b_attn ; q,k,v = split(c)
    present = stack([k, v]) as [B,2,H,S,hd]
    att = softmax(mask(q k^T / sqrt(hd))) @ v
    out = att @ w_proj + b_proj

Sharding: 8 cores = 2 batches x 4 head-groups (4 heads each).
Per core: QKV projection for its 256 q/k/v features (column-sharded
w_attn), attention for its 4 heads, and a partial out-projection
(row-sharded w_proj). Host sums the 4 partials per batch and adds
b_proj exactly.

Layout strategy on-core:
  x[b] is PE-transposed once to xT [D, S] (f32r, 4 transposes batched
  per PSUM bank, DVE eviction).
  Q^T, K^T produced feature-major [feat, S] (f32r), V natural [S, feat].
  Scores are computed directly transposed: S^T[kpos, q] tile per
  128-kpos chunk covering its 384 valid q columns (window 256 spans 3
  q-tiles), so softmax needs no P transposes. Masks: causal triangle on
  the left third, all-valid middle, anti-causal triangle right.
  exp on ACT -> P^T fp16; AV matmul uses V|ones fp16 where columns
  64:128 are all ones, so the softmax denominator lands replicated on
  PSUM partitions 64:128 -> 64-lane reciprocal + multiply on DVE, no
  partition broadcast needed. O^T f32r; out-projection back to natural
  [S, D] layout.
"""

import sys

sys.path.insert(0, "/opt/trn_rl_repo")

import numpy as np

import concourse.bass as bass  # noqa: F401  (bass must import before bacc)
import concourse.mybir as mybir
from concourse import bacc
from concourse.tile import TileContext
from concourse.bass_utils import run_bass_kernel_spmd

F32 = mybir.dt.float32
F32R = mybir.dt.float32r
F16 = mybir.dt.float16

B, S, D = 2, 2048, 1024
N_HEAD = 16
HD = 64
WINDOW = 256
NCORES = 8
HPC = N_HEAD // 4  # heads per core = 4
FPC = HPC * HD  # features per core = 256
NQT = S // 128  # 16 q/kpos tiles
NDC = D // 128  # 8 contraction chunks
SCALE = 1.0 / np.sqrt(HD)

_CACHE = {}


def _build_program():
    nc = bacc.Bacc("TRN2", target_bir_lowering=False, debug=False,
                   num_devices=NCORES)

    # ---- DRAM I/O ----
    xb = nc.dram_tensor("xb", [S, D], F32R, kind="ExternalInput")
    wq = nc.dram_tensor("wq", [128, NDC * FPC], F32R, kind="ExternalInput")
    wk = nc.dram_tensor("wk", [128, NDC * FPC], F32R, kind="ExternalInput")
    wv = nc.dram_tensor("wv", [128, NDC * FPC], F32R, kind="ExternalInput")
    wp = nc.dram_tensor("wp", [128, 2 * D], F32R, kind="ExternalInput")
    bq = nc.dram_tensor("bq", [128, 2], F32, kind="ExternalInput")
    bk = nc.dram_tensor("bk", [128, 2], F32, kind="ExternalInput")
    bv = nc.dram_tensor("bv", [1, FPC], F32R, kind="ExternalInput")
    ident_r = nc.dram_tensor("ident_r", [128, 128], F32R, kind="ExternalInput")
    tri = nc.dram_tensor("tri", [128, 128], F16, kind="ExternalInput")
    atri = nc.dram_tensor("atri", [128, 128], F16, kind="ExternalInput")
    onesr = nc.dram_tensor("onesr", [1, 128], F32R, kind="ExternalInput")

    outp = nc.dram_tensor("outp", [S, D], F32, kind="ExternalOutput")
    pk = nc.dram_tensor("pk", [HPC, S, HD], F32, kind="ExternalOutput")
    pv = nc.dram_tensor("pv", [HPC, S, HD], F32, kind="ExternalOutput")

    Exp = mybir.ActivationFunctionType.Exp
    Ident = mybir.ActivationFunctionType.Identity

    with TileContext(nc) as tc:
        with tc.tile_pool(name="const", bufs=1) as cpool, \
             tc.tile_pool(name="qkv", bufs=1) as qkv, \
             tc.tile_pool(name="ot", bufs=1) as otp:

            # identity first (x transposes depend on it)
            idr_sb = cpool.tile([128, 128], F32R, tag="idr")
            nc.sync.dma_start(out=idr_sb[:], in_=ident_r[:])

            # persistent activations
            qt_sb = [qkv.tile([128, S], F32R, tag=f"qt{i}", name=f"qt{i}")
                     for i in range(2)]
            kt_sb = [qkv.tile([128, S], F32R, tag=f"kt{i}", name=f"kt{i}")
                     for i in range(2)]
            v_sb = qkv.tile([128, NQT * FPC], F32, tag="v")
            # V|ones fp16 per head: head hl block [128, NQT*128];
            # cols 64:128 of each chunk stay 1.0 (denominator trick)
            vb_sb = qkv.tile([128, HPC * NQT * 128], F16, tag="vb")
            ot_sb = [otp.tile([128, S], F32R, tag=f"ot{i}", name=f"ot{i}")
                     for i in range(2)]

            nc.gpsimd.memset(vb_sb[:], 1.0)

            # ---- stage 1: load x tiles, transpose to xT (batched 4/bank) --
            with tc.tile_pool(name="xt", bufs=1) as xtp, \
                 tc.tile_pool(name="xload", bufs=3) as xlp, \
                 tc.tile_pool(name="tp_ps", bufs=2, space="PSUM") as tpps, \
                 tc.tile_pool(name="qk_ps", bufs=4, space="PSUM") as qkps, \
                 tc.tile_pool(name="vp_ps", bufs=2, space="PSUM") as vpps:

                xt_sb = xtp.tile([128, NDC * S], F32R, tag="xt")
                for st in range(NQT):
                    x_t = xlp.tile([128, D], F32R, tag="x")
                    nc.sync.dma_start(out=x_t[:],
                                      in_=xb[st * 128:(st + 1) * 128, :])
                    for dg in range(2):  # two groups of 4 D-chunks
                        ps = tpps.tile([128, 512], F32R, tag="tp")
                        for j in range(4):
                            dc = dg * 4 + j
                            nc.tensor.transpose(
                                ps[:, j * 128:(j + 1) * 128],
                                x_t[:, dc * 128:(dc + 1) * 128], idr_sb[:])
                        # evict 4 chunks at once on DVE
                        dst = xt_sb[:].rearrange(
                            "p (dc s) -> p dc s", dc=NDC)[
                            :, dg * 4:(dg + 1) * 4,
                            st * 128:st * 128 + 128]
                        src = ps[:].rearrange("p (j s) -> p j s", j=4)
                        nc.vector.tensor_copy(dst, src)

                # ---- weights (after x so x loads go first) ----
                wq_sb = cpool.tile([128, NDC * FPC], F32R, tag="wq")
                nc.sync.dma_start(out=wq_sb[:], in_=wq[:])
                wk_sb = cpool.tile([128, NDC * FPC], F32R, tag="wk")
                nc.sync.dma_start(out=wk_sb[:], in_=wk[:])
                wv_sb = cpool.tile([128, NDC * FPC], F32R, tag="wv")
                nc.sync.dma_start(out=wv_sb[:], in_=wv[:])
                wp_sb = cpool.tile([128, 2 * D], F32R, tag="wp")
                nc.sync.dma_start(out=wp_sb[:], in_=wp[:])
                bq_sb = cpool.tile([128, 2], F32, tag="bq")
                nc.sync.dma_start(out=bq_sb[:], in_=bq[:])
                bk_sb = cpool.tile([128, 2], F32, tag="bk")
                nc.sync.dma_start(out=bk_sb[:], in_=bk[:])
                bv_sb = cpool.tile([1, FPC], F32R, tag="bv")
                nc.sync.dma_start(out=bv_sb[:], in_=bv[:])
                tri_sb = cpool.tile([128, 128], F16, tag="tri")
                nc.sync.dma_start(out=tri_sb[:], in_=tri[:])
                atri_sb = cpool.tile([128, 128], F16, tag="atri")
                nc.sync.dma_start(out=atri_sb[:], in_=atri[:])
                on_sb = cpool.tile([1, 128], F32R, tag="on")
                nc.sync.dma_start(out=on_sb[:], in_=onesr[:])

                # ---- stage 2: QKV projection ----
                for w_sb, b_sb, dst in ((wq_sb, bq_sb, qt_sb),
                                        (wk_sb, bk_sb, kt_sb)):
                    for ft in range(2):
                        pss = [qkps.tile([128, 512], F32, tag="qk",
                                         name=f"qkps{ft}_{i}")
                               for i in range(4)]
                        for dc in range(NDC):
                            lhsT = w_sb[:, dc * FPC + ft * 128:
                                        dc * FPC + ft * 128 + 128]
                            for sc in range(4):
                                nc.tensor.matmul(
                                    pss[sc][:], lhsT,
                                    xt_sb[:, dc * S + sc * 512:
                                          dc * S + sc * 512 + 512],
                                    start=(dc == 0), stop=(dc == NDC - 1))
                        for sc in range(4):
                            nc.scalar.activation(
                                dst[ft][:, sc * 512:(sc + 1) * 512],
                                pss[sc][:], Ident,
                                bias=b_sb[:, ft:ft + 1])

                # V natural + fused bias; fp16 V|ones blocks
                for st in range(NQT):
                    vp = vpps.tile([128, FPC], F32, tag="vp")
                    for dc in range(NDC):
                        nc.tensor.matmul(
                            vp[:],
                            xt_sb[:, dc * S + st * 128: dc * S + st * 128 + 128],
                            wv_sb[:, dc * FPC:(dc + 1) * FPC],
                            start=(dc == 0), stop=False)
                    nc.tensor.matmul(vp[:], on_sb[:], bv_sb[:],
                                     start=False, stop=True)
                    nc.scalar.copy(v_sb[:, st * FPC:(st + 1) * FPC], vp[:])
                    # strided fp16 copy into per-head V|ones blocks
                    src = vp[:].rearrange("p (hl c) -> p hl c", hl=HPC)
                    dst3 = vb_sb[:].rearrange("p (hl t) -> p hl t", hl=HPC)[
                        :, :, st * 128: st * 128 + HD]
                    nc.vector.tensor_copy(dst3, src)

            # ---- stage 3: K natural + present outputs ----
            with tc.tile_pool(name="kn", bufs=3) as knp, \
                 tc.tile_pool(name="kn_ps", bufs=3, space="PSUM") as knps:
                for ft in range(2):
                    for st in range(NQT):
                        kp = knps.tile([128, 128], F32R, tag="knp")
                        nc.tensor.transpose(
                            kp[:], kt_sb[ft][:, st * 128:(st + 1) * 128],
                            idr_sb[:])
                        kn = knp.tile([128, 128], F32, tag="kn")
                        nc.scalar.copy(kn[:], kp[:].bitcast(F32))
                        dst = pk[2 * ft: 2 * ft + 2,
                                 st * 128:(st + 1) * 128, :]
                        nc.sync.dma_start(
                            out=dst.rearrange("h s d -> s h d"),
                            in_=kn[:].rearrange("p (h d) -> p h d", h=2))

                for st in range(NQT):
                    nc.sync.dma_start(
                        out=pv[:, st * 128:(st + 1) * 128, :].rearrange(
                            "h s d -> s h d"),
                        in_=v_sb[:, st * FPC:(st + 1) * FPC].rearrange(
                            "p (h d) -> p h d", h=HPC))

            # ---- stage 4: attention per head ----
            with tc.tile_pool(name="pt", bufs=4) as ptp, \
                 tc.tile_pool(name="rs", bufs=4) as rsp, \
                 tc.tile_pool(name="st_ps", bufs=3, space="PSUM") as stps, \
                 tc.tile_pool(name="o_ps", bufs=5, space="PSUM") as ops:
                for hl in range(HPC):
                    ft, po = hl // 2, (hl % 2) * 64
                    kth = kt_sb[ft]
                    qth = qt_sb[ft]
                    oth = ot_sb[ft]
                    vbh = vb_sb[:, hl * NQT * 128:(hl + 1) * NQT * 128]
                    # q-tile t gets kpos chunks t-2 (anti-tri), t-1 (full),
                    # t (causal tri); rolling window of 3 accumulating psums
                    ops_live = [None, None]  # [for c+1, for c+2]
                    for c in range(NQT):
                        qw = min(384, S - c * 128)
                        sp = stps.tile([128, 384], F32, tag="sp")
                        nc.tensor.matmul(
                            sp[:, :qw],
                            kth[po:po + 64, c * 128:(c + 1) * 128],
                            qth[po:po + 64, c * 128:c * 128 + qw],
                            start=True, stop=True)
                        pt = ptp.tile([128, 384], F16, tag="pt")
                        nc.scalar.activation(pt[:, :qw], sp[:, :qw], Exp)
                        nc.vector.tensor_mul(pt[:, 0:128], pt[:, 0:128],
                                             tri_sb[:])
                        if qw > 256:
                            nc.vector.tensor_mul(
                                pt[:, 256:256 + (qw - 256)],
                                pt[:, 256:256 + (qw - 256)],
                                atri_sb[:, 0:qw - 256])
                        lhsT = vbh[:, c * 128:(c + 1) * 128]
                        # finish q-tile c
                        o_cur = ops_live[0]
                        if o_cur is None:
                            o_cur = ops.tile([128, 128], F32, tag="o",
                                             name=f"o{hl}_{c}")
                        nc.tensor.matmul(o_cur[:], lhsT, pt[:, 0:128],
                                         start=(c == 0), stop=True)
                        # middle third -> q-tile c+1
                        if c + 1 < NQT:
                            o1 = ops_live[1]
                            if o1 is None:
                                o1 = ops.tile([128, 128], F32, tag="o",
                                              name=f"o{hl}_{c}m")
                            nc.tensor.matmul(o1[:], lhsT, pt[:, 128:256],
                                             start=(c == 0), stop=False)
                        else:
                            o1 = None
                        # right third -> q-tile c+2
                        if c + 2 < NQT and qw > 256:
                            o2 = ops.tile([128, 128], F32, tag="o",
                                          name=f"o{hl}_{c}r")
                            nc.tensor.matmul(o2[:], lhsT, pt[:, 256:384],
                                             start=True, stop=False)
                        else:
                            o2 = None
                        ops_live = [o1, o2]
                        # normalize q-tile c: denominator replicated on
                        # partitions 64:128 by the ones columns
                        rec = rsp.tile([64, 128], F32, tag="rec")
                        nc.vector.reciprocal(rec[:], o_cur[64:128, :])
                        nc.vector.tensor_mul(
                            oth[po:po + 64, c * 128:(c + 1) * 128],
                            o_cur[0:64, :], rec[:])

            # ---- stage 5: out-projection (partial, natural layout) ----
            with tc.tile_pool(name="osb", bufs=3) as osbp, \
                 tc.tile_pool(name="op_ps", bufs=4, space="PSUM") as opps:
                for st in range(NQT):
                    o_t = osbp.tile([128, D], F32, tag="osb")
                    for half in range(2):
                        op = opps.tile([128, 512], F32, tag="op")
                        nc.tensor.matmul(
                            op[:], ot_sb[0][:, st * 128:(st + 1) * 128],
                            wp_sb[:, half * 512: half * 512 + 512],
                            start=True, stop=False)
                        nc.tensor.matmul(
                            op[:], ot_sb[1][:, st * 128:(st + 1) * 128],
                            wp_sb[:, D + half * 512: D + half * 512 + 512],
                            start=False, stop=True)
                        nc.scalar.copy(o_t[:, half * 512:(half + 1) * 512],
                                       op[:])
                    nc.sync.dma_start(
                        out=outp[st * 128:(st + 1) * 128, :], in_=o_t[:])

    nc.compile()
    return nc


def _prep_in_maps(x, w_attn, b_attn, w_proj):
    """Per-core input dicts (host-side sharding + layout prep)."""
    x = np.ascontiguousarray(np.asarray(x, dtype=np.float32))
    w_attn = np.asarray(w_attn, dtype=np.float32)
    b_attn = np.asarray(b_attn, dtype=np.float32)
    w_proj = np.asarray(w_proj, dtype=np.float32)

    ident = np.eye(128, dtype=np.float32)
    tri = (np.arange(128)[None, :] >= np.arange(128)[:, None]).astype(
        np.float16)
    atri = (np.arange(128)[None, :] < np.arange(128)[:, None]).astype(
        np.float16)
    onesr = np.ones((1, 128), dtype=np.float32)

    def chunk_w(w_cols):  # [D, FPC] -> [128, NDC*FPC]
        return np.ascontiguousarray(
            w_cols.reshape(NDC, 128, FPC).transpose(1, 0, 2).reshape(
                128, NDC * FPC))

    in_maps = []
    for core in range(NCORES):
        b, hg = core // 4, core % 4
        cols = slice(hg * FPC, (hg + 1) * FPC)
        kcols = slice(D + hg * FPC, D + (hg + 1) * FPC)
        vcols = slice(2 * D + hg * FPC, 2 * D + (hg + 1) * FPC)
        rows = slice(hg * FPC, (hg + 1) * FPC)
        in_maps.append({
            "xb": x[b],
            "wq": chunk_w(w_attn[:, cols] * np.float32(SCALE)),
            "wk": chunk_w(w_attn[:, kcols]),
            "wv": chunk_w(w_attn[:, vcols]),
            "wp": np.ascontiguousarray(
                w_proj[rows, :].reshape(2, 128, D).transpose(1, 0, 2).reshape(
                    128, 2 * D)),
            "bq": np.ascontiguousarray(
                (b_attn[cols] * np.float32(SCALE)).reshape(2, 128).T),
            "bk": np.ascontiguousarray(b_attn[kcols].reshape(2, 128).T),
            "bv": b_attn[vcols].reshape(1, FPC).copy(),
            "ident_r": ident,
            "tri": tri,
            "atri": atri,
            "onesr": onesr,
        })
    return in_maps


def kernel(x, w_attn, b_attn, w_proj, b_proj):
    if "nc" not in _CACHE:
        _CACHE["nc"] = _build_program()
    nc = _CACHE["nc"]

    in_maps = _prep_in_maps(x, w_attn, b_attn, w_proj)
    res = run_bass_kernel_spmd(nc, in_maps, core_ids=list(range(NCORES)))

    b_proj = np.asarray(b_proj, dtype=np.float32)
    out = np.zeros((B, S, D), dtype=np.float32)
    present = np.zeros((B, 2, N_HEAD, S, HD), dtype=np.float32)
    for core in range(NCORES):
        b, hg = core // 4, core % 4
        r = res.results[core]
        out[b] += r["outp"]
        present[b, 0, hg * HPC:(hg + 1) * HPC] = r["pk"]
        present[b, 1, hg * HPC:(hg + 1) * HPC] = r["pv"]
    out += b_proj
    return out, present


# revision 6
# speedup vs baseline: 1.1098x; 1.0219x over previous
"""Trainium2 Bass kernel for sliding-window causal attention block.

Reference computation (B=2, S=2048, D=1024, H=16, hd=64, WINDOW=256):
    c = x @ w_attn + b_attn ; q,k,v = split(c)
    present = stack([k, v]) as [B,2,H,S,hd]
    att = softmax(mask(q k^T / sqrt(hd))) @ v
    out = att @ w_proj + b_proj

Sharding: 8 cores = 2 batches x 4 head-groups (4 heads each).
Per core: QKV projection for its 256 q/k/v features (column-sharded
w_attn), attention for its 4 heads, and a partial out-projection
(row-sharded w_proj). Host sums the 4 partials per batch and adds
b_proj exactly.

Layout strategy on-core:
  x[b] is PE-transposed once to xT [D, S] (f32r, 4 transposes batched
  per PSUM bank, DVE eviction).
  Q^T, K^T produced feature-major [feat, S] (f32r), V natural [S, feat].
  Scores are computed directly transposed: S^T[kpos, q] tile per
  128-kpos chunk covering its 384 valid q columns (window 256 spans 3
  q-tiles), so softmax needs no P transposes. Masks: causal triangle on
  the left third, all-valid middle, anti-causal triangle right.
  exp on ACT -> P^T fp16; AV matmul uses V|ones fp16 where columns
  64:128 are all ones, so the softmax denominator lands replicated on
  PSUM partitions 64:128 -> 64-lane reciprocal + multiply on DVE, no
  partition broadcast needed. O^T f32r; out-projection back to natural
  [S, D] layout.
"""

import sys

sys.path.insert(0, "/opt/trn_rl_repo")

import numpy as np

import concourse.bass as bass  # noqa: F401  (bass must import before bacc)
import concourse.mybir as mybir
from concourse import bacc
from concourse.tile import TileContext
from concourse.bass_utils import run_bass_kernel_spmd

F32 = mybir.dt.float32
F32R = mybir.dt.float32r
F16 = mybir.dt.float16

B, S, D = 2, 2048, 1024
N_HEAD = 16
HD = 64
WINDOW = 256
NCORES = 8
HPC = N_HEAD // 4  # heads per core = 4
FPC = HPC * HD  # features per core = 256
NQT = S // 128  # 16 q/kpos tiles
NDC = D // 128  # 8 contraction chunks
SCALE = 1.0 / np.sqrt(HD)

_CACHE = {}


def _build_program():
    nc = bacc.Bacc("TRN2", target_bir_lowering=False, debug=False,
                   num_devices=NCORES)

    # ---- DRAM I/O ----
    xb = nc.dram_tensor("xb", [S, D], F32R, kind="ExternalInput")
    wq = nc.dram_tensor("wq", [128, NDC * FPC], F32R, kind="ExternalInput")
    wk = nc.dram_tensor("wk", [128, NDC * FPC], F32R, kind="ExternalInput")
    wv = nc.dram_tensor("wv", [128, NDC * FPC], F32R, kind="ExternalInput")
    wp = nc.dram_tensor("wp", [128, 2 * D], F32R, kind="ExternalInput")
    bq = nc.dram_tensor("bq", [128, 2], F32, kind="ExternalInput")
    bk = nc.dram_tensor("bk", [128, 2], F32, kind="ExternalInput")
    bv = nc.dram_tensor("bv", [1, FPC], F32R, kind="ExternalInput")
    ident_r = nc.dram_tensor("ident_r", [128, 128], F32R, kind="ExternalInput")
    tri = nc.dram_tensor("tri", [128, 128], F16, kind="ExternalInput")
    atri = nc.dram_tensor("atri", [128, 128], F16, kind="ExternalInput")
    onesr = nc.dram_tensor("onesr", [1, 128], F32R, kind="ExternalInput")

    outp = nc.dram_tensor("outp", [S, D], F32, kind="ExternalOutput")
    pk = nc.dram_tensor("pk", [HPC, S, HD], F32, kind="ExternalOutput")
    pv = nc.dram_tensor("pv", [HPC, S, HD], F32, kind="ExternalOutput")

    Exp = mybir.ActivationFunctionType.Exp
    Ident = mybir.ActivationFunctionType.Identity

    with TileContext(nc) as tc:
        with tc.tile_pool(name="const", bufs=1) as cpool, \
             tc.tile_pool(name="qkv", bufs=1) as qkv, \
             tc.tile_pool(name="ot", bufs=1) as otp:

            # identity first (x transposes depend on it)
            idr_sb = cpool.tile([128, 128], F32R, tag="idr")
            nc.sync.dma_start(out=idr_sb[:], in_=ident_r[:])

            # persistent activations
            qt_sb = [qkv.tile([128, S], F32R, tag=f"qt{i}", name=f"qt{i}")
                     for i in range(2)]
            kt_sb = [qkv.tile([128, S], F32R, tag=f"kt{i}", name=f"kt{i}")
                     for i in range(2)]
            v_sb = qkv.tile([128, NQT * FPC], F32, tag="v")
            # V|ones fp16 per head: head hl block [128, NQT*128];
            # cols 64:128 of each chunk stay 1.0 (denominator trick)
            vb_sb = qkv.tile([128, HPC * NQT * 128], F16, tag="vb")
            ot_sb = [otp.tile([128, S], F32R, tag=f"ot{i}", name=f"ot{i}")
                     for i in range(2)]

            nc.gpsimd.memset(vb_sb[:], 1.0)

            # ---- stage 1: load x tiles, transpose to xT (batched 4/bank) --
            with tc.tile_pool(name="xt", bufs=1) as xtp, \
                 tc.tile_pool(name="xload", bufs=3) as xlp, \
                 tc.tile_pool(name="tp_ps", bufs=2, space="PSUM") as tpps, \
                 tc.tile_pool(name="qk_ps", bufs=4, space="PSUM") as qkps, \
                 tc.tile_pool(name="vp_ps", bufs=2, space="PSUM") as vpps:

                xt_sb = xtp.tile([128, NDC * S], F32R, tag="xt")
                for st in range(NQT):
                    x_t = xlp.tile([128, D], F32R, tag="x")
                    nc.sync.dma_start(out=x_t[:],
                                      in_=xb[st * 128:(st + 1) * 128, :])
                    for dg in range(2):  # two groups of 4 D-chunks
                        ps = tpps.tile([128, 512], F32R, tag="tp")
                        for j in range(4):
                            dc = dg * 4 + j
                            nc.tensor.transpose(
                                ps[:, j * 128:(j + 1) * 128],
                                x_t[:, dc * 128:(dc + 1) * 128], idr_sb[:])
                        # evict 4 chunks at once on DVE
                        dst = xt_sb[:].rearrange(
                            "p (dc s) -> p dc s", dc=NDC)[
                            :, dg * 4:(dg + 1) * 4,
                            st * 128:st * 128 + 128]
                        src = ps[:].rearrange("p (j s) -> p j s", j=4)
                        nc.vector.tensor_copy(dst, src)

                # ---- weights (after x so x loads go first) ----
                wq_sb = cpool.tile([128, NDC * FPC], F32R, tag="wq")
                nc.sync.dma_start(out=wq_sb[:], in_=wq[:])
                wk_sb = cpool.tile([128, NDC * FPC], F32R, tag="wk")
                nc.sync.dma_start(out=wk_sb[:], in_=wk[:])
                wv_sb = cpool.tile([128, NDC * FPC], F32R, tag="wv")
                nc.sync.dma_start(out=wv_sb[:], in_=wv[:])
                wp_sb = cpool.tile([128, 2 * D], F32R, tag="wp")
                nc.sync.dma_start(out=wp_sb[:], in_=wp[:])
                bq_sb = cpool.tile([128, 2], F32, tag="bq")
                nc.sync.dma_start(out=bq_sb[:], in_=bq[:])
                bk_sb = cpool.tile([128, 2], F32, tag="bk")
                nc.sync.dma_start(out=bk_sb[:], in_=bk[:])
                bv_sb = cpool.tile([1, FPC], F32R, tag="bv")
                nc.sync.dma_start(out=bv_sb[:], in_=bv[:])
                tri_sb = cpool.tile([128, 128], F16, tag="tri")
                nc.sync.dma_start(out=tri_sb[:], in_=tri[:])
                atri_sb = cpool.tile([128, 128], F16, tag="atri")
                nc.sync.dma_start(out=atri_sb[:], in_=atri[:])
                on_sb = cpool.tile([1, 128], F32R, tag="on")
                nc.sync.dma_start(out=on_sb[:], in_=onesr[:])

                # ---- stage 2: QKV projection ----
                for w_sb, b_sb, dst in ((wq_sb, bq_sb, qt_sb),
                                        (wk_sb, bk_sb, kt_sb)):
                    for ft in range(2):
                        pss = [qkps.tile([128, 512], F32, tag="qk",
                                         name=f"qkps{ft}_{i}")
                               for i in range(4)]
                        for dc in range(NDC):
                            lhsT = w_sb[:, dc * FPC + ft * 128:
                                        dc * FPC + ft * 128 + 128]
                            for sc in range(4):
                                nc.tensor.matmul(
                                    pss[sc][:], lhsT,
                                    xt_sb[:, dc * S + sc * 512:
                                          dc * S + sc * 512 + 512],
                                    start=(dc == 0), stop=(dc == NDC - 1))
                        for sc in range(4):
                            nc.scalar.activation(
                                dst[ft][:, sc * 512:(sc + 1) * 512],
                                pss[sc][:], Ident,
                                bias=b_sb[:, ft:ft + 1])

                # V natural + fused bias; fp16 V|ones blocks
                for st in range(NQT):
                    vp = vpps.tile([128, FPC], F32, tag="vp")
                    for dc in range(NDC):
                        nc.tensor.matmul(
                            vp[:],
                            xt_sb[:, dc * S + st * 128: dc * S + st * 128 + 128],
                            wv_sb[:, dc * FPC:(dc + 1) * FPC],
                            start=(dc == 0), stop=False)
                    nc.tensor.matmul(vp[:], on_sb[:], bv_sb[:],
                                     start=False, stop=True)
                    nc.scalar.copy(v_sb[:, st * FPC:(st + 1) * FPC], vp[:])
                    # strided fp16 copy into per-head V|ones blocks
                    src = vp[:].rearrange("p (hl c) -> p hl c", hl=HPC)
                    dst3 = vb_sb[:].rearrange("p (hl t) -> p hl t", hl=HPC)[
                        :, :, st * 128: st * 128 + HD]
                    nc.vector.tensor_copy(dst3, src)

            # ---- stage 3: K natural + present outputs ----
            with tc.tile_pool(name="kn", bufs=3) as knp, \
                 tc.tile_pool(name="kn_ps", bufs=3, space="PSUM") as knps:
                for ft in range(2):
                    for st in range(NQT):
                        kp = knps.tile([128, 128], F32R, tag="knp")
                        nc.tensor.transpose(
                            kp[:], kt_sb[ft][:, st * 128:(st + 1) * 128],
                            idr_sb[:])
                        kn = knp.tile([128, 128], F32, tag="kn")
                        nc.scalar.copy(kn[:], kp[:].bitcast(F32))
                        dst = pk[2 * ft: 2 * ft + 2,
                                 st * 128:(st + 1) * 128, :]
                        nc.sync.dma_start(
                            out=dst.rearrange("h s d -> s h d"),
                            in_=kn[:].rearrange("p (h d) -> p h d", h=2))

                for st in range(NQT):
                    nc.sync.dma_start(
                        out=pv[:, st * 128:(st + 1) * 128, :].rearrange(
                            "h s d -> s h d"),
                        in_=v_sb[:, st * FPC:(st + 1) * FPC].rearrange(
                            "p (h d) -> p h d", h=HPC))

            # ---- stage 4: attention per head (software-pipelined) ----
            with tc.tile_pool(name="pt", bufs=4) as ptp, \
                 tc.tile_pool(name="rs", bufs=4) as rsp, \
                 tc.tile_pool(name="st_ps", bufs=3, space="PSUM") as stps, \
                 tc.tile_pool(name="o_ps", bufs=5, space="PSUM") as ops:
                for hl in range(HPC):
                    ft, po = hl // 2, (hl % 2) * 64
                    kth = kt_sb[ft]
                    qth = qt_sb[ft]
                    oth = ot_sb[ft]
                    vbh = vb_sb[:, hl * NQT * 128:(hl + 1) * NQT * 128]

                    pts = [None] * NQT
                    osums = [None] * NQT

                    def qk_exp_mask(c, hl=hl, kth=kth, qth=qth):
                        qw = min(384, S - c * 128)
                        sp = stps.tile([128, 384], F32, tag="sp",
                                       name=f"sp{hl}_{c}")
                        nc.tensor.matmul(
                            sp[:, :qw],
                            kth[po:po + 64, c * 128:(c + 1) * 128],
                            qth[po:po + 64, c * 128:c * 128 + qw],
                            start=True, stop=True)
                        pt = ptp.tile([128, 384], F16, tag="pt",
                                      name=f"pt{hl}_{c}")
                        nc.scalar.activation(pt[:, :qw], sp[:, :qw], Exp)
                        nc.vector.tensor_mul(pt[:, 0:128], pt[:, 0:128],
                                             tri_sb[:])
                        if qw > 256:
                            nc.vector.tensor_mul(
                                pt[:, 256:256 + (qw - 256)],
                                pt[:, 256:256 + (qw - 256)],
                                atri_sb[:, 0:qw - 256])
                        pts[c] = pt

                    def osum(c, create, hl=hl):
                        if osums[c] is None and create:
                            osums[c] = ops.tile([128, 128], F32, tag="o",
                                                name=f"o{hl}_{c}")
                        return osums[c]

                    # prologue: 2 chunks of QK/exp/mask ahead
                    qk_exp_mask(0)
                    qk_exp_mask(1)
                    for c in range(NQT):
                        if c + 2 < NQT:
                            qk_exp_mask(c + 2)
                        pt = pts[c]
                        qw = min(384, S - c * 128)
                        lhsT = vbh[:, c * 128:(c + 1) * 128]
                        o_cur = osum(c, True)
                        nc.tensor.matmul(o_cur[:], lhsT, pt[:, 0:128],
                                         start=(c == 0), stop=True)
                        if c + 1 < NQT:
                            nc.tensor.matmul(osum(c + 1, True)[:], lhsT,
                                             pt[:, 128:256],
                                             start=(c == 0), stop=False)
                        if c + 2 < NQT and qw > 256:
                            nc.tensor.matmul(osum(c + 2, True)[:], lhsT,
                                             pt[:, 256:384],
                                             start=True, stop=False)
                        # normalize q-tile c: denominator replicated on
                        # partitions 64:128 by the ones columns
                        rec = rsp.tile([64, 128], F32, tag="rec",
                                       name=f"rec{hl}_{c}")
                        nc.vector.reciprocal(rec[:], o_cur[64:128, :])
                        nc.vector.tensor_mul(
                            oth[po:po + 64, c * 128:(c + 1) * 128],
                            o_cur[0:64, :], rec[:])
                        pts[c] = None

            # ---- stage 5: out-projection (partial, natural layout) ----
            with tc.tile_pool(name="osb", bufs=3) as osbp, \
                 tc.tile_pool(name="op_ps", bufs=4, space="PSUM") as opps:
                for st in range(NQT):
                    o_t = osbp.tile([128, D], F32, tag="osb")
                    for half in range(2):
                        op = opps.tile([128, 512], F32, tag="op")
                        nc.tensor.matmul(
                            op[:], ot_sb[0][:, st * 128:(st + 1) * 128],
                            wp_sb[:, half * 512: half * 512 + 512],
                            start=True, stop=False)
                        nc.tensor.matmul(
                            op[:], ot_sb[1][:, st * 128:(st + 1) * 128],
                            wp_sb[:, D + half * 512: D + half * 512 + 512],
                            start=False, stop=True)
                        nc.scalar.copy(o_t[:, half * 512:(half + 1) * 512],
                                       op[:])
                    nc.sync.dma_start(
                        out=outp[st * 128:(st + 1) * 128, :], in_=o_t[:])

    nc.compile()
    return nc


def _prep_in_maps(x, w_attn, b_attn, w_proj):
    """Per-core input dicts (host-side sharding + layout prep)."""
    x = np.ascontiguousarray(np.asarray(x, dtype=np.float32))
    w_attn = np.asarray(w_attn, dtype=np.float32)
    b_attn = np.asarray(b_attn, dtype=np.float32)
    w_proj = np.asarray(w_proj, dtype=np.float32)

    ident = np.eye(128, dtype=np.float32)
    tri = (np.arange(128)[None, :] >= np.arange(128)[:, None]).astype(
        np.float16)
    atri = (np.arange(128)[None, :] < np.arange(128)[:, None]).astype(
        np.float16)
    onesr = np.ones((1, 128), dtype=np.float32)

    def chunk_w(w_cols):  # [D, FPC] -> [128, NDC*FPC]
        return np.ascontiguousarray(
            w_cols.reshape(NDC, 128, FPC).transpose(1, 0, 2).reshape(
                128, NDC * FPC))

    in_maps = []
    for core in range(NCORES):
        b, hg = core // 4, core % 4
        cols = slice(hg * FPC, (hg + 1) * FPC)
        kcols = slice(D + hg * FPC, D + (hg + 1) * FPC)
        vcols = slice(2 * D + hg * FPC, 2 * D + (hg + 1) * FPC)
        rows = slice(hg * FPC, (hg + 1) * FPC)
        in_maps.append({
            "xb": x[b],
            "wq": chunk_w(w_attn[:, cols] * np.float32(SCALE)),
            "wk": chunk_w(w_attn[:, kcols]),
            "wv": chunk_w(w_attn[:, vcols]),
            "wp": np.ascontiguousarray(
                w_proj[rows, :].reshape(2, 128, D).transpose(1, 0, 2).reshape(
                    128, 2 * D)),
            "bq": np.ascontiguousarray(
                (b_attn[cols] * np.float32(SCALE)).reshape(2, 128).T),
            "bk": np.ascontiguousarray(b_attn[kcols].reshape(2, 128).T),
            "bv": b_attn[vcols].reshape(1, FPC).copy(),
            "ident_r": ident,
            "tri": tri,
            "atri": atri,
            "onesr": onesr,
        })
    return in_maps


def kernel(x, w_attn, b_attn, w_proj, b_proj):
    if "nc" not in _CACHE:
        _CACHE["nc"] = _build_program()
    nc = _CACHE["nc"]

    in_maps = _prep_in_maps(x, w_attn, b_attn, w_proj)
    res = run_bass_kernel_spmd(nc, in_maps, core_ids=list(range(NCORES)))

    b_proj = np.asarray(b_proj, dtype=np.float32)
    out = np.zeros((B, S, D), dtype=np.float32)
    present = np.zeros((B, 2, N_HEAD, S, HD), dtype=np.float32)
    for core in range(NCORES):
        b, hg = core // 4, core % 4
        r = res.results[core]
        out[b] += r["outp"]
        present[b, 0, hg * HPC:(hg + 1) * HPC] = r["pk"]
        present[b, 1, hg * HPC:(hg + 1) * HPC] = r["pv"]
    out += b_proj
    return out, present


# revision 7
# speedup vs baseline: 1.2587x; 1.1342x over previous
"""Trainium2 Bass kernel for sliding-window causal attention block.

Reference computation (B=2, S=2048, D=1024, H=16, hd=64, WINDOW=256):
    c = x @ w_attn + b_attn ; q,k,v = split(c)
    present = stack([k, v]) as [B,2,H,S,hd]
    att = softmax(mask(q k^T / sqrt(hd))) @ v
    out = att @ w_proj + b_proj

Sharding: 8 cores = 2 batches x 4 head-groups (4 heads each).
Per core: QKV projection for its 256 q/k/v features (column-sharded
w_attn), attention for its 4 heads, and a partial out-projection
(row-sharded w_proj). Host sums the 4 partials per batch and adds
b_proj exactly.

Layout strategy on-core:
  x[b] is PE-transposed once to xT [D, S] (f32r, 4 transposes batched
  per PSUM bank, eviction split across DVE/ACT).
  Q^T, K^T produced feature-major [feat, S] (f32r), V natural [S, feat].
  Scores are computed directly transposed: S^T[kpos, q] tile per
  128-kpos chunk covering its 384 valid q columns (window 256 spans 3
  q-tiles), so softmax needs no P transposes. Masks: causal triangle on
  the left third, all-valid middle, anti-causal triangle right.
  exp on ACT -> P^T fp16; AV matmul uses V|ones fp16 where columns
  64:128 are all ones, so the softmax denominator lands replicated on
  PSUM partitions 64:128 -> 64-lane reciprocal + multiply on DVE.
  O^T f32r; out-projection back to natural [S, D] layout.
  DMAs are batched aggressively (the SP sequencer pays ~0.6us per DMA
  instruction): 8 input DMAs for x, 1 for all of wq/wk/wv, single
  per-head DMAs for the present k/v outputs, 2-S-tile DMAs for out.
"""

import sys

sys.path.insert(0, "/opt/trn_rl_repo")

import numpy as np

import concourse.bass as bass  # noqa: F401  (bass must import before bacc)
import concourse.mybir as mybir
from concourse import bacc
from concourse.tile import TileContext
from concourse.bass_utils import run_bass_kernel_spmd

F32 = mybir.dt.float32
F32R = mybir.dt.float32r
F16 = mybir.dt.float16

B, S, D = 2, 2048, 1024
N_HEAD = 16
HD = 64
WINDOW = 256
NCORES = 8
HPC = N_HEAD // 4  # heads per core = 4
FPC = HPC * HD  # features per core = 256
NQT = S // 128  # 16 q/kpos tiles
NDC = D // 128  # 8 contraction chunks
SCALE = 1.0 / np.sqrt(HD)

_CACHE = {}


def _build_program():
    nc = bacc.Bacc("TRN2", target_bir_lowering=False, debug=False,
                   num_devices=NCORES)

    # ---- DRAM I/O ----
    xb = nc.dram_tensor("xb", [S, D], F32R, kind="ExternalInput")
    # wq | wk | wv, each pre-chunked to [128, NDC*FPC]
    wqkv = nc.dram_tensor("wqkv", [128, 3 * NDC * FPC], F32R,
                          kind="ExternalInput")
    wp = nc.dram_tensor("wp", [128, 2 * D], F32R, kind="ExternalInput")
    bqk = nc.dram_tensor("bqk", [128, 4], F32, kind="ExternalInput")
    bv = nc.dram_tensor("bv", [1, FPC], F32R, kind="ExternalInput")
    ident_r = nc.dram_tensor("ident_r", [128, 128], F32R, kind="ExternalInput")
    tris = nc.dram_tensor("tris", [128, 256], F16, kind="ExternalInput")
    onesr = nc.dram_tensor("onesr", [1, 128], F32R, kind="ExternalInput")

    outp = nc.dram_tensor("outp", [S, D], F32, kind="ExternalOutput")
    pk = nc.dram_tensor("pk", [HPC, S, HD], F32, kind="ExternalOutput")
    pv = nc.dram_tensor("pv", [HPC, S, HD], F32, kind="ExternalOutput")

    Exp = mybir.ActivationFunctionType.Exp
    Ident = mybir.ActivationFunctionType.Identity

    with TileContext(nc) as tc:
        with tc.tile_pool(name="const", bufs=1) as cpool, \
             tc.tile_pool(name="qkv", bufs=1) as qkv, \
             tc.tile_pool(name="ot", bufs=1) as otp:

            # identity first (x transposes depend on it)
            idr_sb = cpool.tile([128, 128], F32R, tag="idr")
            nc.sync.dma_start(out=idr_sb[:], in_=ident_r[:])

            # persistent activations
            qt_sb = [qkv.tile([128, S], F32R, tag=f"qt{i}", name=f"qt{i}")
                     for i in range(2)]
            kt_sb = [qkv.tile([128, S], F32R, tag=f"kt{i}", name=f"kt{i}")
                     for i in range(2)]
            v_sb = qkv.tile([128, NQT * FPC], F32, tag="v")
            # V|ones fp16 per head: head hl block [128, NQT*128];
            # cols 64:128 of each chunk stay 1.0 (denominator trick)
            vb_sb = qkv.tile([128, HPC * NQT * 128], F16, tag="vb")
            ot_sb = [otp.tile([128, S], F32R, tag=f"ot{i}", name=f"ot{i}")
                     for i in range(2)]

            nc.gpsimd.memset(vb_sb[:], 1.0)

            # ---- stage 1+2: x load/transpose + QKV projection ----
            with tc.tile_pool(name="xt", bufs=1) as xtp, \
                 tc.tile_pool(name="xload", bufs=2) as xlp, \
                 tc.tile_pool(name="tp_ps", bufs=2, space="PSUM") as tpps, \
                 tc.tile_pool(name="qk_ps", bufs=4, space="PSUM") as qkps, \
                 tc.tile_pool(name="vp_ps", bufs=2, space="PSUM") as vpps:

                xt_sb = xtp.tile([128, NDC * S], F32R, tag="xt")
                xt3 = xt_sb[:].rearrange("p (dc s) -> p dc s", dc=NDC)
                for st2 in range(NQT // 2):
                    x_t = xlp.tile([128, 2 * D], F32R, tag="x")
                    src = xb[st2 * 256:(st2 + 1) * 256, :].rearrange(
                        "(i p) d -> p i d", p=128)
                    nc.sync.dma_start(out=x_t[:].rearrange(
                        "p (i d) -> p i d", i=2), in_=src)
                    for i in range(2):
                        st = st2 * 2 + i
                        for dg in range(2):  # two groups of 4 D-chunks
                            ps = tpps.tile([128, 512], F32R, tag="tp")
                            for j in range(4):
                                dc = dg * 4 + j
                                nc.tensor.transpose(
                                    ps[:, j * 128:(j + 1) * 128],
                                    x_t[:, i * D + dc * 128:
                                        i * D + (dc + 1) * 128], idr_sb[:])
                            dst = xt3[:, dg * 4:(dg + 1) * 4,
                                      st * 128:st * 128 + 128]
                            srcp = ps[:].rearrange("p (j s) -> p j s", j=4)
                            # alternate eviction engine
                            if (st + dg) % 2 == 0:
                                nc.vector.tensor_copy(dst, srcp)
                            else:
                                nc.scalar.copy(dst, srcp)

                # ---- weights (after x so x loads go first) ----
                wqkv_sb = cpool.tile([128, 3 * NDC * FPC], F32R, tag="wqkv")
                nc.sync.dma_start(out=wqkv_sb[:], in_=wqkv[:])
                wq_sb = wqkv_sb[:, 0:NDC * FPC]
                wk_sb = wqkv_sb[:, NDC * FPC:2 * NDC * FPC]
                wv_sb = wqkv_sb[:, 2 * NDC * FPC:3 * NDC * FPC]
                bqk_sb = cpool.tile([128, 4], F32, tag="bqk")
                nc.sync.dma_start(out=bqk_sb[:], in_=bqk[:])
                bv_sb = cpool.tile([1, FPC], F32R, tag="bv")
                nc.sync.dma_start(out=bv_sb[:], in_=bv[:])
                tris_sb = cpool.tile([128, 256], F16, tag="tris")
                nc.sync.dma_start(out=tris_sb[:], in_=tris[:])
                tri_sb = tris_sb[:, 0:128]
                atri_sb = tris_sb[:, 128:256]
                on_sb = cpool.tile([1, 128], F32R, tag="on")
                nc.sync.dma_start(out=on_sb[:], in_=onesr[:])

                # Q^T / K^T (feature-major). Q evicts on ACT (+bias),
                # K evicts on DVE (+bias).
                for wi, (w_sb, dst) in enumerate(((wq_sb, qt_sb),
                                                  (wk_sb, kt_sb))):
                    for ft in range(2):
                        pss = [qkps.tile([128, 512], F32, tag="qk",
                                         name=f"qkps{wi}{ft}_{i}")
                               for i in range(4)]
                        for dc in range(NDC):
                            lhsT = w_sb[:, dc * FPC + ft * 128:
                                        dc * FPC + ft * 128 + 128]
                            for sc in range(4):
                                nc.tensor.matmul(
                                    pss[sc][:], lhsT,
                                    xt_sb[:, dc * S + sc * 512:
                                          dc * S + sc * 512 + 512],
                                    start=(dc == 0), stop=(dc == NDC - 1))
                        bias_ap = bqk_sb[:, 2 * wi + ft: 2 * wi + ft + 1]
                        for sc in range(4):
                            dslc = dst[ft][:, sc * 512:(sc + 1) * 512]
                            if wi == 0:
                                nc.scalar.activation(dslc, pss[sc][:], Ident,
                                                     bias=bias_ap)
                            else:
                                nc.vector.tensor_scalar_add(dslc, pss[sc][:],
                                                            bias_ap)

                # V natural + fused bias; fp16 V|ones blocks
                for st in range(NQT):
                    vp = vpps.tile([128, FPC], F32, tag="vp")
                    for dc in range(NDC):
                        nc.tensor.matmul(
                            vp[:],
                            xt_sb[:, dc * S + st * 128: dc * S + st * 128 + 128],
                            wv_sb[:, dc * FPC:(dc + 1) * FPC],
                            start=(dc == 0), stop=False)
                    nc.tensor.matmul(vp[:], on_sb[:], bv_sb[:],
                                     start=False, stop=True)
                    nc.scalar.copy(v_sb[:, st * FPC:(st + 1) * FPC], vp[:])
                    # strided fp16 copy into per-head V|ones blocks
                    srcv = vp[:].rearrange("p (hl c) -> p hl c", hl=HPC)
                    dst3 = vb_sb[:].rearrange("p (hl t) -> p hl t", hl=HPC)[
                        :, :, st * 128: st * 128 + HD]
                    nc.vector.tensor_copy(dst3, srcv)

            # pools that reuse the space freed by xt: wp, K-natural
            # collector, attention transients, out staging
            with tc.tile_pool(name="late", bufs=1) as late, \
                 tc.tile_pool(name="pt", bufs=4) as ptp, \
                 tc.tile_pool(name="rs", bufs=4) as rsp, \
                 tc.tile_pool(name="osb", bufs=2) as osbp:

                wp_sb = late.tile([128, 2 * D], F32R, tag="wp")
                nc.sync.dma_start(out=wp_sb[:], in_=wp[:])
                kn_sb = late.tile([128, 2 * NQT * 128], F32, tag="kn")

                # ---- stage 3: K natural + present outputs ----
                with tc.tile_pool(name="kn_ps", bufs=3, space="PSUM") as knps:
                    for ft in range(2):
                        for st in range(NQT):
                            kp = knps.tile([128, 128], F32R, tag="knp")
                            nc.tensor.transpose(
                                kp[:], kt_sb[ft][:, st * 128:(st + 1) * 128],
                                idr_sb[:])
                            dstk = kn_sb[:, ft * S + st * 128:
                                         ft * S + st * 128 + 128]
                            if st % 2 == 0:
                                nc.scalar.copy(dstk, kp[:].bitcast(F32))
                            else:
                                nc.vector.tensor_copy(dstk, kp[:].bitcast(F32))
                    # one DMA per head
                    kn4 = kn_sb[:].rearrange(
                        "p (ft st h d) -> p ft st h d", ft=2, st=NQT, h=2)
                    for ft in range(2):
                        for h2 in range(2):
                            nc.sync.dma_start(
                                out=pk[2 * ft + h2, :, :].rearrange(
                                    "(st p) d -> p st d", p=128),
                                in_=kn4[:, ft, :, h2, :])
                    v4 = v_sb[:].rearrange(
                        "p (st hl d) -> p st hl d", st=NQT, hl=HPC)
                    for hl in range(HPC):
                        nc.sync.dma_start(
                            out=pv[hl, :, :].rearrange(
                                "(st p) d -> p st d", p=128),
                            in_=v4[:, :, hl, :])

                # ---- stage 4: attention per head (software-pipelined) ----
                with tc.tile_pool(name="st_ps", bufs=3, space="PSUM") as stps, \
                     tc.tile_pool(name="o_ps", bufs=5, space="PSUM") as ops:
                    for hl in range(HPC):
                        ft, po = hl // 2, (hl % 2) * 64
                        kth = kt_sb[ft]
                        qth = qt_sb[ft]
                        oth = ot_sb[ft]
                        vbh = vb_sb[:, hl * NQT * 128:(hl + 1) * NQT * 128]

                        pts = [None] * NQT
                        osums = [None] * NQT

                        def qk_exp_mask(c, hl=hl, kth=kth, qth=qth, po=po,
                                        pts=pts):
                            qw = min(384, S - c * 128)
                            sp = stps.tile([128, 384], F32, tag="sp",
                                           name=f"sp{hl}_{c}")
                            nc.tensor.matmul(
                                sp[:, :qw],
                                kth[po:po + 64, c * 128:(c + 1) * 128],
                                qth[po:po + 64, c * 128:c * 128 + qw],
                                start=True, stop=True)
                            pt = ptp.tile([128, 384], F16, tag="pt",
                                          name=f"pt{hl}_{c}")
                            nc.scalar.activation(pt[:, :qw], sp[:, :qw], Exp)
                            nc.vector.tensor_mul(pt[:, 0:128], pt[:, 0:128],
                                                 tri_sb)
                            if qw > 256:
                                nc.vector.tensor_mul(
                                    pt[:, 256:256 + (qw - 256)],
                                    pt[:, 256:256 + (qw - 256)],
                                    atri_sb[:, 0:qw - 256])
                            pts[c] = pt

                        def osum(c, hl=hl, osums=osums):
                            if osums[c] is None:
                                osums[c] = ops.tile([128, 128], F32, tag="o",
                                                    name=f"o{hl}_{c}")
                            return osums[c]

                        # prologue: 2 chunks of QK/exp/mask ahead
                        qk_exp_mask(0)
                        qk_exp_mask(1)
                        for c in range(NQT):
                            if c + 2 < NQT:
                                qk_exp_mask(c + 2)
                            pt = pts[c]
                            qw = min(384, S - c * 128)
                            lhsT = vbh[:, c * 128:(c + 1) * 128]
                            o_cur = osum(c)
                            nc.tensor.matmul(o_cur[:], lhsT, pt[:, 0:128],
                                             start=(c == 0), stop=True)
                            if c + 1 < NQT:
                                nc.tensor.matmul(osum(c + 1)[:], lhsT,
                                                 pt[:, 128:256],
                                                 start=(c == 0), stop=False)
                            if c + 2 < NQT and qw > 256:
                                nc.tensor.matmul(osum(c + 2)[:], lhsT,
                                                 pt[:, 256:384],
                                                 start=True, stop=False)
                            # normalize q-tile c: denominator replicated on
                            # partitions 64:128 by the ones columns
                            rec = rsp.tile([64, 128], F32, tag="rec",
                                           name=f"rec{hl}_{c}")
                            nc.vector.reciprocal(rec[:], o_cur[64:128, :])
                            nc.vector.tensor_mul(
                                oth[po:po + 64, c * 128:(c + 1) * 128],
                                o_cur[0:64, :], rec[:])
                            pts[c] = None

                # ---- stage 5: out-projection (partial, natural layout) ----
                with tc.tile_pool(name="op_ps", bufs=4, space="PSUM") as opps:
                    for st2 in range(NQT // 2):
                        o_t = osbp.tile([128, 2 * D], F32, tag="osb")
                        for i in range(2):
                            st = st2 * 2 + i
                            for half in range(2):
                                op = opps.tile([128, 512], F32, tag="op")
                                nc.tensor.matmul(
                                    op[:],
                                    ot_sb[0][:, st * 128:(st + 1) * 128],
                                    wp_sb[:, half * 512: half * 512 + 512],
                                    start=True, stop=False)
                                nc.tensor.matmul(
                                    op[:],
                                    ot_sb[1][:, st * 128:(st + 1) * 128],
                                    wp_sb[:, D + half * 512: D + half * 512 + 512],
                                    start=False, stop=True)
                                dsl = o_t[:, i * D + half * 512:
                                          i * D + (half + 1) * 512]
                                if half == 0:
                                    nc.scalar.copy(dsl, op[:])
                                else:
                                    nc.vector.tensor_copy(dsl, op[:])
                        nc.sync.dma_start(
                            out=outp[st2 * 256:(st2 + 1) * 256, :].rearrange(
                                "(i p) d -> p i d", p=128),
                            in_=o_t[:].rearrange("p (i d) -> p i d", i=2))

    nc.compile()
    return nc


def _prep_in_maps(x, w_attn, b_attn, w_proj):
    """Per-core input dicts (host-side sharding + layout prep)."""
    x = np.ascontiguousarray(np.asarray(x, dtype=np.float32))
    w_attn = np.asarray(w_attn, dtype=np.float32)
    b_attn = np.asarray(b_attn, dtype=np.float32)
    w_proj = np.asarray(w_proj, dtype=np.float32)

    ident = np.eye(128, dtype=np.float32)
    tri = (np.arange(128)[None, :] >= np.arange(128)[:, None]).astype(
        np.float16)
    atri = (np.arange(128)[None, :] < np.arange(128)[:, None]).astype(
        np.float16)
    tris = np.ascontiguousarray(np.concatenate([tri, atri], axis=1))
    onesr = np.ones((1, 128), dtype=np.float32)

    def chunk_w(w_cols):  # [D, FPC] -> [128, NDC*FPC]
        return w_cols.reshape(NDC, 128, FPC).transpose(1, 0, 2).reshape(
            128, NDC * FPC)

    in_maps = []
    for core in range(NCORES):
        b, hg = core // 4, core % 4
        cols = slice(hg * FPC, (hg + 1) * FPC)
        kcols = slice(D + hg * FPC, D + (hg + 1) * FPC)
        vcols = slice(2 * D + hg * FPC, 2 * D + (hg + 1) * FPC)
        rows = slice(hg * FPC, (hg + 1) * FPC)
        wqkv = np.concatenate(
            [chunk_w(w_attn[:, cols] * np.float32(SCALE)),
             chunk_w(w_attn[:, kcols]),
             chunk_w(w_attn[:, vcols])], axis=1)
        bqk = np.stack(
            [(b_attn[cols] * np.float32(SCALE)).reshape(2, 128)[0],
             (b_attn[cols] * np.float32(SCALE)).reshape(2, 128)[1],
             b_attn[kcols].reshape(2, 128)[0],
             b_attn[kcols].reshape(2, 128)[1]], axis=1)
        in_maps.append({
            "xb": x[b],
            "wqkv": np.ascontiguousarray(wqkv),
            "wp": np.ascontiguousarray(
                w_proj[rows, :].reshape(2, 128, D).transpose(1, 0, 2).reshape(
                    128, 2 * D)),
            "bqk": np.ascontiguousarray(bqk),
            "bv": b_attn[vcols].reshape(1, FPC).copy(),
            "ident_r": ident,
            "tris": tris,
            "onesr": onesr,
        })
    return in_maps


def kernel(x, w_attn, b_attn, w_proj, b_proj):
    if "nc" not in _CACHE:
        _CACHE["nc"] = _build_program()
    nc = _CACHE["nc"]

    in_maps = _prep_in_maps(x, w_attn, b_attn, w_proj)
    res = run_bass_kernel_spmd(nc, in_maps, core_ids=list(range(NCORES)))

    b_proj = np.asarray(b_proj, dtype=np.float32)
    out = np.zeros((B, S, D), dtype=np.float32)
    present = np.zeros((B, 2, N_HEAD, S, HD), dtype=np.float32)
    for core in range(NCORES):
        b, hg = core // 4, core % 4
        r = res.results[core]
        out[b] += r["outp"]
        present[b, 0, hg * HPC:(hg + 1) * HPC] = r["pk"]
        present[b, 1, hg * HPC:(hg + 1) * HPC] = r["pv"]
    out += b_proj
    return out, present


# revision 8
# speedup vs baseline: 1.3060x; 1.0376x over previous
"""Trainium2 Bass kernel for sliding-window causal attention block.

Reference computation (B=2, S=2048, D=1024, H=16, hd=64, WINDOW=256):
    c = x @ w_attn + b_attn ; q,k,v = split(c)
    present = stack([k, v]) as [B,2,H,S,hd]
    att = softmax(mask(q k^T / sqrt(hd))) @ v
    out = att @ w_proj + b_proj

Sharding: 8 cores = 2 batches x 4 head-groups (4 heads each).
Per core: QKV projection for its 256 q/k/v features (column-sharded
w_attn), attention for its 4 heads, and a partial out-projection
(row-sharded w_proj). Host sums the 4 partials per batch and adds
b_proj exactly.

Layout strategy on-core:
  x[b] is PE-transposed once to xT [D, S] (f32r, 4 transposes batched
  per PSUM bank, eviction split across DVE/ACT).
  Q^T, K^T produced feature-major [feat, S] (f32r), V natural [S, feat].
  Scores are computed directly transposed: S^T[kpos, q] tile per
  128-kpos chunk covering its 384 valid q columns (window 256 spans 3
  q-tiles), so softmax needs no P transposes. Masks: causal triangle on
  the left third, all-valid middle, anti-causal triangle right.
  exp on ACT -> P^T fp16; AV matmul uses V|ones fp16 where columns
  64:128 are all ones, so the softmax denominator lands replicated on
  PSUM partitions 64:128 -> 64-lane reciprocal + multiply on DVE.
  O^T f32r; out-projection back to natural [S, D] layout.
  DMAs are batched aggressively (the SP sequencer pays ~0.6us per DMA
  instruction): 8 input DMAs for x, 1 for all of wq/wk/wv, single
  per-head DMAs for the present k/v outputs, 2-S-tile DMAs for out.
"""

import sys

sys.path.insert(0, "/opt/trn_rl_repo")

import numpy as np

import concourse.bass as bass  # noqa: F401  (bass must import before bacc)
import concourse.mybir as mybir
from concourse import bacc
from concourse.tile import TileContext
from concourse.bass_utils import run_bass_kernel_spmd

F32 = mybir.dt.float32
F32R = mybir.dt.float32r
F16 = mybir.dt.float16

B, S, D = 2, 2048, 1024
N_HEAD = 16
HD = 64
WINDOW = 256
NCORES = 8
HPC = N_HEAD // 4  # heads per core = 4
FPC = HPC * HD  # features per core = 256
NQT = S // 128  # 16 q/kpos tiles
NDC = D // 128  # 8 contraction chunks
SCALE = 1.0 / np.sqrt(HD)

_CACHE = {}


def _build_program():
    nc = bacc.Bacc("TRN2", target_bir_lowering=False, debug=False,
                   num_devices=NCORES)

    # ---- DRAM I/O ----
    xb = nc.dram_tensor("xb", [S, D], F32R, kind="ExternalInput")
    # wq | wk | wv, each pre-chunked to [128, NDC*FPC]
    wqkv = nc.dram_tensor("wqkv", [128, 3 * NDC * FPC], F32R,
                          kind="ExternalInput")
    wp = nc.dram_tensor("wp", [128, 2 * D], F32R, kind="ExternalInput")
    bqk = nc.dram_tensor("bqk", [128, 4], F32, kind="ExternalInput")
    bv = nc.dram_tensor("bv", [1, FPC], F32R, kind="ExternalInput")
    ident_r = nc.dram_tensor("ident_r", [128, 128], F32R, kind="ExternalInput")
    tris = nc.dram_tensor("tris", [128, 256], F16, kind="ExternalInput")
    onesr = nc.dram_tensor("onesr", [1, 128], F32R, kind="ExternalInput")

    outp = nc.dram_tensor("outp", [S, D], F32, kind="ExternalOutput")
    pk = nc.dram_tensor("pk", [HPC, S, HD], F32, kind="ExternalOutput")
    pv = nc.dram_tensor("pv", [HPC, S, HD], F32, kind="ExternalOutput")

    Exp = mybir.ActivationFunctionType.Exp
    Ident = mybir.ActivationFunctionType.Identity

    with TileContext(nc) as tc:
        with tc.tile_pool(name="const", bufs=1) as cpool, \
             tc.tile_pool(name="qkv", bufs=1) as qkv, \
             tc.tile_pool(name="ot", bufs=1) as otp:

            # identity first (x transposes depend on it)
            idr_sb = cpool.tile([128, 128], F32R, tag="idr")
            nc.sync.dma_start(out=idr_sb[:], in_=ident_r[:])

            # persistent activations
            qt_sb = [qkv.tile([128, S], F32R, tag=f"qt{i}", name=f"qt{i}")
                     for i in range(2)]
            kt_sb = [qkv.tile([128, S], F32R, tag=f"kt{i}", name=f"kt{i}")
                     for i in range(2)]
            v_sb = qkv.tile([128, NQT * FPC], F32, tag="v")
            # V|ones fp16 per head: head hl block [128, NQT*128];
            # cols 64:128 of each chunk stay 1.0 (denominator trick)
            vb_sb = qkv.tile([128, HPC * NQT * 128], F16, tag="vb")
            ot_sb = [otp.tile([128, S], F32R, tag=f"ot{i}", name=f"ot{i}")
                     for i in range(2)]

            nc.gpsimd.memset(vb_sb[:], 1.0)

            # ---- stage 1+2: x load/transpose + QKV projection ----
            with tc.tile_pool(name="xt", bufs=1) as xtp, \
                 tc.tile_pool(name="xload", bufs=2) as xlp, \
                 tc.tile_pool(name="tp_ps", bufs=2, space="PSUM") as tpps, \
                 tc.tile_pool(name="qk_ps", bufs=4, space="PSUM") as qkps, \
                 tc.tile_pool(name="vp_ps", bufs=2, space="PSUM") as vpps:

                xt_sb = xtp.tile([128, NDC * S], F32R, tag="xt")
                xt3 = xt_sb[:].rearrange("p (dc s) -> p dc s", dc=NDC)
                for st2 in range(NQT // 2):
                    x_t = xlp.tile([128, 2 * D], F32R, tag="x")
                    src = xb[st2 * 256:(st2 + 1) * 256, :].rearrange(
                        "(i p) d -> p i d", p=128)
                    nc.sync.dma_start(out=x_t[:].rearrange(
                        "p (i d) -> p i d", i=2), in_=src)
                    for i in range(2):
                        st = st2 * 2 + i
                        for dg in range(2):  # two groups of 4 D-chunks
                            ps = tpps.tile([128, 512], F32R, tag="tp")
                            for j in range(4):
                                dc = dg * 4 + j
                                nc.tensor.transpose(
                                    ps[:, j * 128:(j + 1) * 128],
                                    x_t[:, i * D + dc * 128:
                                        i * D + (dc + 1) * 128], idr_sb[:])
                            dst = xt3[:, dg * 4:(dg + 1) * 4,
                                      st * 128:st * 128 + 128]
                            srcp = ps[:].rearrange("p (j s) -> p j s", j=4)
                            # alternate eviction engine
                            if (st + dg) % 2 == 0:
                                nc.vector.tensor_copy(dst, srcp)
                            else:
                                nc.scalar.copy(dst, srcp)

                # ---- weights (after x so x loads go first) ----
                wqkv_sb = cpool.tile([128, 3 * NDC * FPC], F32R, tag="wqkv")
                nc.sync.dma_start(out=wqkv_sb[:], in_=wqkv[:])
                wq_sb = wqkv_sb[:, 0:NDC * FPC]
                wk_sb = wqkv_sb[:, NDC * FPC:2 * NDC * FPC]
                wv_sb = wqkv_sb[:, 2 * NDC * FPC:3 * NDC * FPC]
                bqk_sb = cpool.tile([128, 4], F32, tag="bqk")
                nc.sync.dma_start(out=bqk_sb[:], in_=bqk[:])
                bv_sb = cpool.tile([1, FPC], F32R, tag="bv")
                nc.sync.dma_start(out=bv_sb[:], in_=bv[:])
                tris_sb = cpool.tile([128, 256], F16, tag="tris")
                nc.sync.dma_start(out=tris_sb[:], in_=tris[:])
                tri_sb = tris_sb[:, 0:128]
                atri_sb = tris_sb[:, 128:256]
                on_sb = cpool.tile([1, 128], F32R, tag="on")
                nc.sync.dma_start(out=on_sb[:], in_=onesr[:])

                # Q^T / K^T (feature-major). Q evicts on ACT (+bias),
                # K evicts on DVE (+bias).
                for wi, (w_sb, dst) in enumerate(((wq_sb, qt_sb),
                                                  (wk_sb, kt_sb))):
                    for ft in range(2):
                        pss = [qkps.tile([128, 512], F32, tag="qk",
                                         name=f"qkps{wi}{ft}_{i}")
                               for i in range(4)]
                        for dc in range(NDC):
                            lhsT = w_sb[:, dc * FPC + ft * 128:
                                        dc * FPC + ft * 128 + 128]
                            for sc in range(4):
                                nc.tensor.matmul(
                                    pss[sc][:], lhsT,
                                    xt_sb[:, dc * S + sc * 512:
                                          dc * S + sc * 512 + 512],
                                    start=(dc == 0), stop=(dc == NDC - 1))
                        bias_ap = bqk_sb[:, 2 * wi + ft: 2 * wi + ft + 1]
                        for sc in range(4):
                            dslc = dst[ft][:, sc * 512:(sc + 1) * 512]
                            if wi == 0:
                                nc.scalar.activation(dslc, pss[sc][:], Ident,
                                                     bias=bias_ap)
                            else:
                                nc.vector.tensor_scalar_add(dslc, pss[sc][:],
                                                            bias_ap)

                # V natural + fused bias; fp16 V|ones blocks
                for st in range(NQT):
                    vp = vpps.tile([128, FPC], F32, tag="vp")
                    for dc in range(NDC):
                        nc.tensor.matmul(
                            vp[:],
                            xt_sb[:, dc * S + st * 128: dc * S + st * 128 + 128],
                            wv_sb[:, dc * FPC:(dc + 1) * FPC],
                            start=(dc == 0), stop=False)
                    nc.tensor.matmul(vp[:], on_sb[:], bv_sb[:],
                                     start=False, stop=True)
                    nc.scalar.copy(v_sb[:, st * FPC:(st + 1) * FPC], vp[:])
                    # strided fp16 copy into per-head V|ones blocks
                    srcv = vp[:].rearrange("p (hl c) -> p hl c", hl=HPC)
                    dst3 = vb_sb[:].rearrange("p (hl t) -> p hl t", hl=HPC)[
                        :, :, st * 128: st * 128 + HD]
                    if st % 2 == 0:
                        nc.vector.tensor_copy(dst3, srcv)
                    else:
                        nc.scalar.copy(dst3, srcv)

            # pools that reuse the space freed by xt: wp, K-natural
            # collector, attention transients, out staging
            with tc.tile_pool(name="late", bufs=1) as late, \
                 tc.tile_pool(name="pt", bufs=4) as ptp, \
                 tc.tile_pool(name="rs", bufs=4) as rsp, \
                 tc.tile_pool(name="osb", bufs=2) as osbp:

                wp_sb = late.tile([128, 2 * D], F32R, tag="wp")
                nc.sync.dma_start(out=wp_sb[:], in_=wp[:])
                kn_sb = late.tile([128, 2 * NQT * 128], F32, tag="kn")

                # ---- stage 3: K natural + present outputs ----
                with tc.tile_pool(name="kn_ps", bufs=3, space="PSUM") as knps:
                    for ft in range(2):
                        for st in range(NQT):
                            kp = knps.tile([128, 128], F32R, tag="knp")
                            nc.tensor.transpose(
                                kp[:], kt_sb[ft][:, st * 128:(st + 1) * 128],
                                idr_sb[:])
                            dstk = kn_sb[:, ft * S + st * 128:
                                         ft * S + st * 128 + 128]
                            if st % 2 == 0:
                                nc.scalar.copy(dstk, kp[:].bitcast(F32))
                            else:
                                nc.vector.tensor_copy(dstk, kp[:].bitcast(F32))
                    # one DMA per head
                    kn4 = kn_sb[:].rearrange(
                        "p (ft st h d) -> p ft st h d", ft=2, st=NQT, h=2)
                    for ft in range(2):
                        for h2 in range(2):
                            nc.sync.dma_start(
                                out=pk[2 * ft + h2, :, :].rearrange(
                                    "(st p) d -> p st d", p=128),
                                in_=kn4[:, ft, :, h2, :])
                    v4 = v_sb[:].rearrange(
                        "p (st hl d) -> p st hl d", st=NQT, hl=HPC)
                    for hl in range(HPC):
                        nc.sync.dma_start(
                            out=pv[hl, :, :].rearrange(
                                "(st p) d -> p st d", p=128),
                            in_=v4[:, :, hl, :])

                # ---- stage 4: attention per head (software-pipelined).
                # The last head also interleaves the out-projection so PE
                # fills exp-latency stalls with useful work. ----
                def attention_head(hl, stps, ops, norm_lag, per_chunk_hook):
                    ft, po = hl // 2, (hl % 2) * 64
                    kth = kt_sb[ft]
                    qth = qt_sb[ft]
                    oth = ot_sb[ft]
                    vbh = vb_sb[:, hl * NQT * 128:(hl + 1) * NQT * 128]

                    pts = [None] * NQT
                    osums = [None] * NQT

                    def qk_exp_mask(c):
                        qw = min(384, S - c * 128)
                        sp = stps.tile([128, 384], F32, tag="sp",
                                       name=f"sp{hl}_{c}")
                        nc.tensor.matmul(
                            sp[:, :qw],
                            kth[po:po + 64, c * 128:(c + 1) * 128],
                            qth[po:po + 64, c * 128:c * 128 + qw],
                            start=True, stop=True)
                        pt = ptp.tile([128, 384], F16, tag="pt",
                                      name=f"pt{hl}_{c}")
                        nc.scalar.activation(pt[:, :qw], sp[:, :qw], Exp)
                        nc.vector.tensor_mul(pt[:, 0:128], pt[:, 0:128],
                                             tri_sb)
                        if qw > 256:
                            nc.vector.tensor_mul(
                                pt[:, 256:256 + (qw - 256)],
                                pt[:, 256:256 + (qw - 256)],
                                atri_sb[:, 0:qw - 256])
                        pts[c] = pt

                    def osum(c):
                        if osums[c] is None:
                            osums[c] = ops.tile([128, 128], F32, tag="o",
                                                name=f"o{hl}_{c}")
                        return osums[c]

                    def normalize(c):
                        o_cur = osums[c]
                        rec = rsp.tile([64, 128], F32, tag="rec",
                                       name=f"rec{hl}_{c}")
                        nc.vector.reciprocal(rec[:], o_cur[64:128, :])
                        nc.vector.tensor_mul(
                            oth[po:po + 64, c * 128:(c + 1) * 128],
                            o_cur[0:64, :], rec[:])
                        osums[c] = None

                    qk_exp_mask(0)
                    qk_exp_mask(1)
                    for c in range(NQT):
                        if c + 2 < NQT:
                            qk_exp_mask(c + 2)
                        pt = pts[c]
                        qw = min(384, S - c * 128)
                        lhsT = vbh[:, c * 128:(c + 1) * 128]
                        nc.tensor.matmul(osum(c)[:], lhsT, pt[:, 0:128],
                                         start=(c == 0), stop=True)
                        if c + 1 < NQT:
                            nc.tensor.matmul(osum(c + 1)[:], lhsT,
                                             pt[:, 128:256],
                                             start=(c == 0), stop=False)
                        if c + 2 < NQT and qw > 256:
                            nc.tensor.matmul(osum(c + 2)[:], lhsT,
                                             pt[:, 256:384],
                                             start=True, stop=False)
                        # normalization lags the AV matmuls so the DVE
                        # stream never round-trips against PE
                        if c >= norm_lag:
                            normalize(c - norm_lag)
                            if per_chunk_hook is not None:
                                per_chunk_hook(c - norm_lag)
                        pts[c] = None
                    for c in range(NQT - norm_lag, NQT):
                        normalize(c)
                        if per_chunk_hook is not None:
                            per_chunk_hook(c)

                osb_state = {}

                def outproj_tile(st, opps):
                    # called once per q-tile st (in order) after all heads
                    # normalized it
                    st2, i = st // 2, st % 2
                    if i == 0:
                        osb_state["t"] = osbp.tile([128, 2 * D], F32,
                                                   tag="osb",
                                                   name=f"osb{st2}")
                    o_t = osb_state["t"]
                    for half in range(2):
                        op = opps.tile([128, 512], F32, tag="op",
                                       name=f"op{st}_{half}")
                        nc.tensor.matmul(
                            op[:], ot_sb[0][:, st * 128:(st + 1) * 128],
                            wp_sb[:, half * 512: half * 512 + 512],
                            start=True, stop=False)
                        nc.tensor.matmul(
                            op[:], ot_sb[1][:, st * 128:(st + 1) * 128],
                            wp_sb[:, D + half * 512: D + half * 512 + 512],
                            start=False, stop=True)
                        dsl = o_t[:, i * D + half * 512:
                                  i * D + (half + 1) * 512]
                        if half == 0:
                            nc.scalar.copy(dsl, op[:])
                        else:
                            nc.vector.tensor_copy(dsl, op[:])
                    if i == 1:
                        nc.sync.dma_start(
                            out=outp[st2 * 256:(st2 + 1) * 256, :].rearrange(
                                "(j p) d -> p j d", p=128),
                            in_=o_t[:].rearrange("p (j d) -> p j d", j=2))

                with tc.tile_pool(name="st_ps", bufs=3, space="PSUM") as stps, \
                     tc.tile_pool(name="o_ps", bufs=5, space="PSUM") as ops:
                    for hl in range(HPC - 1):
                        attention_head(hl, stps, ops, 2, None)

                with tc.tile_pool(name="st_ps2", bufs=2, space="PSUM") as stps, \
                     tc.tile_pool(name="o_ps2", bufs=4, space="PSUM") as ops, \
                     tc.tile_pool(name="op_ps", bufs=2, space="PSUM") as opps:
                    attention_head(HPC - 1, stps, ops, 1,
                                   lambda st: outproj_tile(st, opps))

    nc.compile()
    return nc


def _prep_in_maps(x, w_attn, b_attn, w_proj):
    """Per-core input dicts (host-side sharding + layout prep)."""
    x = np.ascontiguousarray(np.asarray(x, dtype=np.float32))
    w_attn = np.asarray(w_attn, dtype=np.float32)
    b_attn = np.asarray(b_attn, dtype=np.float32)
    w_proj = np.asarray(w_proj, dtype=np.float32)

    ident = np.eye(128, dtype=np.float32)
    tri = (np.arange(128)[None, :] >= np.arange(128)[:, None]).astype(
        np.float16)
    atri = (np.arange(128)[None, :] < np.arange(128)[:, None]).astype(
        np.float16)
    tris = np.ascontiguousarray(np.concatenate([tri, atri], axis=1))
    onesr = np.ones((1, 128), dtype=np.float32)

    def chunk_w(w_cols):  # [D, FPC] -> [128, NDC*FPC]
        return w_cols.reshape(NDC, 128, FPC).transpose(1, 0, 2).reshape(
            128, NDC * FPC)

    in_maps = []
    for core in range(NCORES):
        b, hg = core // 4, core % 4
        cols = slice(hg * FPC, (hg + 1) * FPC)
        kcols = slice(D + hg * FPC, D + (hg + 1) * FPC)
        vcols = slice(2 * D + hg * FPC, 2 * D + (hg + 1) * FPC)
        rows = slice(hg * FPC, (hg + 1) * FPC)
        wqkv = np.concatenate(
            [chunk_w(w_attn[:, cols] * np.float32(SCALE)),
             chunk_w(w_attn[:, kcols]),
             chunk_w(w_attn[:, vcols])], axis=1)
        bqk = np.stack(
            [(b_attn[cols] * np.float32(SCALE)).reshape(2, 128)[0],
             (b_attn[cols] * np.float32(SCALE)).reshape(2, 128)[1],
             b_attn[kcols].reshape(2, 128)[0],
             b_attn[kcols].reshape(2, 128)[1]], axis=1)
        in_maps.append({
            "xb": x[b],
            "wqkv": np.ascontiguousarray(wqkv),
            "wp": np.ascontiguousarray(
                w_proj[rows, :].reshape(2, 128, D).transpose(1, 0, 2).reshape(
                    128, 2 * D)),
            "bqk": np.ascontiguousarray(bqk),
            "bv": b_attn[vcols].reshape(1, FPC).copy(),
            "ident_r": ident,
            "tris": tris,
            "onesr": onesr,
        })
    return in_maps


def kernel(x, w_attn, b_attn, w_proj, b_proj):
    if "nc" not in _CACHE:
        _CACHE["nc"] = _build_program()
    nc = _CACHE["nc"]

    in_maps = _prep_in_maps(x, w_attn, b_attn, w_proj)
    res = run_bass_kernel_spmd(nc, in_maps, core_ids=list(range(NCORES)))

    b_proj = np.asarray(b_proj, dtype=np.float32)
    out = np.zeros((B, S, D), dtype=np.float32)
    present = np.zeros((B, 2, N_HEAD, S, HD), dtype=np.float32)
    for core in range(NCORES):
        b, hg = core // 4, core % 4
        r = res.results[core]
        out[b] += r["outp"]
        present[b, 0, hg * HPC:(hg + 1) * HPC] = r["pk"]
        present[b, 1, hg * HPC:(hg + 1) * HPC] = r["pv"]
    out += b_proj
    return out, present


# revision 9
# speedup vs baseline: 1.4243x; 1.0906x over previous
"""Trainium2 Bass kernel for sliding-window causal attention block.

Reference computation (B=2, S=2048, D=1024, H=16, hd=64, WINDOW=256):
    c = x @ w_attn + b_attn ; q,k,v = split(c)
    present = stack([k, v]) as [B,2,H,S,hd]
    att = softmax(mask(q k^T / sqrt(hd))) @ v
    out = att @ w_proj + b_proj

Sharding: 8 cores = 2 batches x 4 head-groups (4 heads each).
Per core: QKV projection for its 256 q/k/v features (column-sharded
w_attn), attention for its 4 heads, and a partial out-projection
(row-sharded w_proj). Host sums the 4 partials per batch and adds
b_proj exactly.

Layout strategy on-core:
  x[b] is PE-transposed once to xT [D, S] (f32r, 4 transposes batched
  per PSUM bank, eviction split across DVE/ACT).
  Q^T, K^T produced feature-major [feat, S] (f32r), V natural [S, feat].
  Scores are computed directly transposed: S^T[kpos, q] tile per
  128-kpos chunk covering its 384 valid q columns (window 256 spans 3
  q-tiles), so softmax needs no P transposes. Masks: causal triangle on
  the left third, all-valid middle, anti-causal triangle right.
  exp on ACT -> P^T fp16; AV matmul uses V|ones fp16 where columns
  64:128 are all ones, so the softmax denominator lands replicated on
  PSUM partitions 64:128 -> 64-lane reciprocal + multiply on DVE.
  O^T f32r; out-projection back to natural [S, D] layout.
  DMAs are batched aggressively (the SP sequencer pays ~0.6us per DMA
  instruction): 8 input DMAs for x, 1 for all of wq/wk/wv, single
  per-head DMAs for the present k/v outputs, 2-S-tile DMAs for out.
"""

import sys

sys.path.insert(0, "/opt/trn_rl_repo")

import numpy as np

import concourse.bass as bass  # noqa: F401  (bass must import before bacc)
import concourse.mybir as mybir
from concourse import bacc
from concourse.tile import TileContext
from concourse.bass_utils import run_bass_kernel_spmd

F32 = mybir.dt.float32
F32R = mybir.dt.float32r
F16 = mybir.dt.float16

B, S, D = 2, 2048, 1024
N_HEAD = 16
HD = 64
WINDOW = 256
NCORES = 8
HPC = N_HEAD // 4  # heads per core = 4
FPC = HPC * HD  # features per core = 256
NQT = S // 128  # 16 q/kpos tiles
NDC = D // 128  # 8 contraction chunks
SCALE = 1.0 / np.sqrt(HD)

_CACHE = {}


def _build_program():
    nc = bacc.Bacc("TRN2", target_bir_lowering=False, debug=False,
                   num_devices=NCORES)

    # ---- DRAM I/O ----
    xb = nc.dram_tensor("xb", [S, D], F32R, kind="ExternalInput")
    # wq | wk | wv, each pre-chunked to [128, NDC*FPC]
    wqkv = nc.dram_tensor("wqkv", [128, 3 * NDC * FPC], F32R,
                          kind="ExternalInput")
    wp = nc.dram_tensor("wp", [128, 2 * D], F32R, kind="ExternalInput")
    bqk = nc.dram_tensor("bqk", [128, 4], F32, kind="ExternalInput")
    bv = nc.dram_tensor("bv", [1, FPC], F32R, kind="ExternalInput")
    ident_r = nc.dram_tensor("ident_r", [128, 128], F32R, kind="ExternalInput")
    madd = nc.dram_tensor("madd", [128, 384], F32R, kind="ExternalInput")
    onesr = nc.dram_tensor("onesr", [1, 128], F32R, kind="ExternalInput")

    outp = nc.dram_tensor("outp", [S, D], F32, kind="ExternalOutput")
    pk = nc.dram_tensor("pk", [HPC, S, HD], F32, kind="ExternalOutput")
    pv = nc.dram_tensor("pv", [HPC, S, HD], F32, kind="ExternalOutput")

    Exp = mybir.ActivationFunctionType.Exp
    Ident = mybir.ActivationFunctionType.Identity

    with TileContext(nc) as tc:
        with tc.tile_pool(name="const", bufs=1) as cpool, \
             tc.tile_pool(name="qkv", bufs=1) as qkv, \
             tc.tile_pool(name="ot", bufs=1) as otp:

            # identity first (x transposes depend on it)
            idr_sb = cpool.tile([128, 128], F32R, tag="idr")
            nc.sync.dma_start(out=idr_sb[:], in_=ident_r[:])

            # persistent activations
            qt_sb = [qkv.tile([128, S], F32R, tag=f"qt{i}", name=f"qt{i}")
                     for i in range(2)]
            kt_sb = [qkv.tile([128, S], F32R, tag=f"kt{i}", name=f"kt{i}")
                     for i in range(2)]
            v_sb = qkv.tile([128, NQT * FPC], F32, tag="v")
            # V|ones fp16 per head: head hl block [128, NQT*128];
            # cols 64:128 of each chunk stay 1.0 (denominator trick)
            vb_sb = qkv.tile([128, HPC * NQT * 128], F16, tag="vb")
            ot_sb = [otp.tile([128, S], F32R, tag=f"ot{i}", name=f"ot{i}")
                     for i in range(2)]

            nc.gpsimd.memset(vb_sb[:], 1.0)

            # ---- stage 1+2: x load/transpose + QKV projection ----
            with tc.tile_pool(name="xt", bufs=1) as xtp, \
                 tc.tile_pool(name="xload", bufs=2) as xlp, \
                 tc.tile_pool(name="tp_ps", bufs=2, space="PSUM") as tpps, \
                 tc.tile_pool(name="qk_ps", bufs=4, space="PSUM") as qkps, \
                 tc.tile_pool(name="vp_ps", bufs=2, space="PSUM") as vpps:

                xt_sb = xtp.tile([128, NDC * S], F32R, tag="xt")
                xt3 = xt_sb[:].rearrange("p (dc s) -> p dc s", dc=NDC)
                for st2 in range(NQT // 2):
                    x_t = xlp.tile([128, 2 * D], F32R, tag="x")
                    src = xb[st2 * 256:(st2 + 1) * 256, :].rearrange(
                        "(i p) d -> p i d", p=128)
                    nc.sync.dma_start(out=x_t[:].rearrange(
                        "p (i d) -> p i d", i=2), in_=src)
                    for i in range(2):
                        st = st2 * 2 + i
                        for dg in range(2):  # two groups of 4 D-chunks
                            ps = tpps.tile([128, 512], F32R, tag="tp")
                            for j in range(4):
                                dc = dg * 4 + j
                                nc.tensor.transpose(
                                    ps[:, j * 128:(j + 1) * 128],
                                    x_t[:, i * D + dc * 128:
                                        i * D + (dc + 1) * 128], idr_sb[:])
                            dst = xt3[:, dg * 4:(dg + 1) * 4,
                                      st * 128:st * 128 + 128]
                            srcp = ps[:].rearrange("p (j s) -> p j s", j=4)
                            # alternate eviction engine
                            if (st + dg) % 2 == 0:
                                nc.vector.tensor_copy(dst, srcp)
                            else:
                                nc.scalar.copy(dst, srcp)

                # ---- weights (after x so x loads go first) ----
                wqkv_sb = cpool.tile([128, 3 * NDC * FPC], F32R, tag="wqkv")
                nc.sync.dma_start(out=wqkv_sb[:], in_=wqkv[:])
                wq_sb = wqkv_sb[:, 0:NDC * FPC]
                wk_sb = wqkv_sb[:, NDC * FPC:2 * NDC * FPC]
                wv_sb = wqkv_sb[:, 2 * NDC * FPC:3 * NDC * FPC]
                bqk_sb = cpool.tile([128, 4], F32, tag="bqk")
                nc.sync.dma_start(out=bqk_sb[:], in_=bqk[:])
                bv_sb = cpool.tile([1, FPC], F32R, tag="bv")
                nc.sync.dma_start(out=bv_sb[:], in_=bv[:])
                madd_sb = cpool.tile([128, 384], F32R, tag="madd")
                nc.sync.dma_start(out=madd_sb[:], in_=madd[:])
                on_sb = cpool.tile([1, 128], F32R, tag="on")
                nc.sync.dma_start(out=on_sb[:], in_=onesr[:])

                # Q^T / K^T (feature-major). Q evicts on ACT (+bias),
                # K evicts on DVE (+bias).
                for wi, (w_sb, dst) in enumerate(((wq_sb, qt_sb),
                                                  (wk_sb, kt_sb))):
                    for ft in range(2):
                        pss = [qkps.tile([128, 512], F32, tag="qk",
                                         name=f"qkps{wi}{ft}_{i}")
                               for i in range(4)]
                        for dc in range(NDC):
                            lhsT = w_sb[:, dc * FPC + ft * 128:
                                        dc * FPC + ft * 128 + 128]
                            for sc in range(4):
                                nc.tensor.matmul(
                                    pss[sc][:], lhsT,
                                    xt_sb[:, dc * S + sc * 512:
                                          dc * S + sc * 512 + 512],
                                    start=(dc == 0), stop=(dc == NDC - 1))
                        bias_ap = bqk_sb[:, 2 * wi + ft: 2 * wi + ft + 1]
                        for sc in range(4):
                            dslc = dst[ft][:, sc * 512:(sc + 1) * 512]
                            if wi == 0:
                                nc.scalar.activation(dslc, pss[sc][:], Ident,
                                                     bias=bias_ap)
                            else:
                                nc.vector.tensor_scalar_add(dslc, pss[sc][:],
                                                            bias_ap)

                # V natural + fused bias; fp16 V|ones blocks
                for st in range(NQT):
                    vp = vpps.tile([128, FPC], F32, tag="vp")
                    for dc in range(NDC):
                        nc.tensor.matmul(
                            vp[:],
                            xt_sb[:, dc * S + st * 128: dc * S + st * 128 + 128],
                            wv_sb[:, dc * FPC:(dc + 1) * FPC],
                            start=(dc == 0), stop=False)
                    nc.tensor.matmul(vp[:], on_sb[:], bv_sb[:],
                                     start=False, stop=True)
                    nc.scalar.copy(v_sb[:, st * FPC:(st + 1) * FPC], vp[:])
                    # strided fp16 copy into per-head V|ones blocks
                    srcv = vp[:].rearrange("p (hl c) -> p hl c", hl=HPC)
                    dst3 = vb_sb[:].rearrange("p (hl t) -> p hl t", hl=HPC)[
                        :, :, st * 128: st * 128 + HD]
                    if st % 2 == 0:
                        nc.vector.tensor_copy(dst3, srcv)
                    else:
                        nc.scalar.copy(dst3, srcv)

            # pools that reuse the space freed by xt: wp, K-natural
            # collector, attention transients, out staging
            with tc.tile_pool(name="late", bufs=1) as late, \
                 tc.tile_pool(name="pt", bufs=4) as ptp, \
                 tc.tile_pool(name="rs", bufs=4) as rsp, \
                 tc.tile_pool(name="osb", bufs=2) as osbp:

                wp_sb = late.tile([128, 2 * D], F32R, tag="wp")
                nc.sync.dma_start(out=wp_sb[:], in_=wp[:])
                kn_sb = late.tile([128, 2 * NQT * 128], F32, tag="kn")

                # ---- stage 3: K natural + present outputs ----
                with tc.tile_pool(name="kn_ps", bufs=3, space="PSUM") as knps:
                    for ft in range(2):
                        for st in range(NQT):
                            kp = knps.tile([128, 128], F32R, tag="knp")
                            nc.tensor.transpose(
                                kp[:], kt_sb[ft][:, st * 128:(st + 1) * 128],
                                idr_sb[:])
                            dstk = kn_sb[:, ft * S + st * 128:
                                         ft * S + st * 128 + 128]
                            if st % 2 == 0:
                                nc.scalar.copy(dstk, kp[:].bitcast(F32))
                            else:
                                nc.vector.tensor_copy(dstk, kp[:].bitcast(F32))
                    # one DMA per head
                    kn4 = kn_sb[:].rearrange(
                        "p (ft st h d) -> p ft st h d", ft=2, st=NQT, h=2)
                    for ft in range(2):
                        for h2 in range(2):
                            nc.sync.dma_start(
                                out=pk[2 * ft + h2, :, :].rearrange(
                                    "(st p) d -> p st d", p=128),
                                in_=kn4[:, ft, :, h2, :])
                    v4 = v_sb[:].rearrange(
                        "p (st hl d) -> p st hl d", st=NQT, hl=HPC)
                    for hl in range(HPC):
                        nc.sync.dma_start(
                            out=pv[hl, :, :].rearrange(
                                "(st p) d -> p st d", p=128),
                            in_=v4[:, :, hl, :])

                # ---- stage 4: attention per head (software-pipelined).
                # The last head also interleaves the out-projection so PE
                # fills exp-latency stalls with useful work. ----
                def attention_head(hl, stps, ops, norm_lag, per_chunk_hook):
                    ft, po = hl // 2, (hl % 2) * 64
                    kth = kt_sb[ft]
                    qth = qt_sb[ft]
                    oth = ot_sb[ft]
                    vbh = vb_sb[:, hl * NQT * 128:(hl + 1) * NQT * 128]

                    pts = [None] * NQT
                    osums = [None] * NQT

                    def qk_exp_mask(c):
                        qw = min(384, S - c * 128)
                        sp = stps.tile([128, 384], F32, tag="sp",
                                       name=f"sp{hl}_{c}")
                        nc.tensor.matmul(
                            sp[:, :qw],
                            kth[po:po + 64, c * 128:(c + 1) * 128],
                            qth[po:po + 64, c * 128:c * 128 + qw],
                            start=True, stop=False)
                        # additive mask (-1e30 off-window) via I.T @ madd
                        nc.tensor.matmul(
                            sp[:, :qw], idr_sb[:], madd_sb[:, :qw],
                            start=False, stop=True)
                        pt = ptp.tile([128, 384], F16, tag="pt",
                                      name=f"pt{hl}_{c}")
                        nc.scalar.activation(pt[:, :qw], sp[:, :qw], Exp)
                        pts[c] = pt

                    def osum(c):
                        if osums[c] is None:
                            osums[c] = ops.tile([128, 128], F32, tag="o",
                                                name=f"o{hl}_{c}")
                        return osums[c]

                    def normalize(c):
                        o_cur = osums[c]
                        rec = rsp.tile([64, 128], F32, tag="rec",
                                       name=f"rec{hl}_{c}")
                        nc.vector.reciprocal(rec[:], o_cur[64:128, :])
                        nc.vector.tensor_mul(
                            oth[po:po + 64, c * 128:(c + 1) * 128],
                            o_cur[0:64, :], rec[:])
                        osums[c] = None

                    qk_exp_mask(0)
                    qk_exp_mask(1)
                    for c in range(NQT):
                        if c + 2 < NQT:
                            qk_exp_mask(c + 2)
                        pt = pts[c]
                        qw = min(384, S - c * 128)
                        lhsT = vbh[:, c * 128:(c + 1) * 128]
                        nc.tensor.matmul(osum(c)[:], lhsT, pt[:, 0:128],
                                         start=(c == 0), stop=True)
                        if c + 1 < NQT:
                            nc.tensor.matmul(osum(c + 1)[:], lhsT,
                                             pt[:, 128:256],
                                             start=(c == 0), stop=False)
                        if c + 2 < NQT and qw > 256:
                            nc.tensor.matmul(osum(c + 2)[:], lhsT,
                                             pt[:, 256:384],
                                             start=True, stop=False)
                        # normalization lags the AV matmuls so the DVE
                        # stream never round-trips against PE
                        if c >= norm_lag:
                            normalize(c - norm_lag)
                            if per_chunk_hook is not None:
                                per_chunk_hook(c - norm_lag)
                        pts[c] = None
                    for c in range(NQT - norm_lag, NQT):
                        normalize(c)
                        if per_chunk_hook is not None:
                            per_chunk_hook(c)

                osb_state = {}

                def outproj_tile(st, opps):
                    # called once per q-tile st (in order) after all heads
                    # normalized it
                    st2, i = st // 2, st % 2
                    if i == 0:
                        osb_state["t"] = osbp.tile([128, 2 * D], F32,
                                                   tag="osb",
                                                   name=f"osb{st2}")
                    o_t = osb_state["t"]
                    for half in range(2):
                        op = opps.tile([128, 512], F32, tag="op",
                                       name=f"op{st}_{half}")
                        nc.tensor.matmul(
                            op[:], ot_sb[0][:, st * 128:(st + 1) * 128],
                            wp_sb[:, half * 512: half * 512 + 512],
                            start=True, stop=False)
                        nc.tensor.matmul(
                            op[:], ot_sb[1][:, st * 128:(st + 1) * 128],
                            wp_sb[:, D + half * 512: D + half * 512 + 512],
                            start=False, stop=True)
                        dsl = o_t[:, i * D + half * 512:
                                  i * D + (half + 1) * 512]
                        if half == 0:
                            nc.scalar.copy(dsl, op[:])
                        else:
                            nc.vector.tensor_copy(dsl, op[:])
                    if i == 1:
                        nc.sync.dma_start(
                            out=outp[st2 * 256:(st2 + 1) * 256, :].rearrange(
                                "(j p) d -> p j d", p=128),
                            in_=o_t[:].rearrange("p (j d) -> p j d", j=2))

                with tc.tile_pool(name="st_ps", bufs=3, space="PSUM") as stps, \
                     tc.tile_pool(name="o_ps", bufs=5, space="PSUM") as ops:
                    for hl in range(HPC - 1):
                        attention_head(hl, stps, ops, 2, None)

                with tc.tile_pool(name="st_ps2", bufs=2, space="PSUM") as stps, \
                     tc.tile_pool(name="o_ps2", bufs=4, space="PSUM") as ops, \
                     tc.tile_pool(name="op_ps", bufs=2, space="PSUM") as opps:
                    attention_head(HPC - 1, stps, ops, 1,
                                   lambda st: outproj_tile(st, opps))

    nc.compile()
    return nc


def _prep_in_maps(x, w_attn, b_attn, w_proj):
    """Per-core input dicts (host-side sharding + layout prep)."""
    x = np.ascontiguousarray(np.asarray(x, dtype=np.float32))
    w_attn = np.asarray(w_attn, dtype=np.float32)
    b_attn = np.asarray(b_attn, dtype=np.float32)
    w_proj = np.asarray(w_proj, dtype=np.float32)

    ident = np.eye(128, dtype=np.float32)
    ql = np.arange(128)[None, :]
    kl = np.arange(128)[:, None]
    neg = np.float32(-1e30)
    madd = np.concatenate(
        [np.where(ql >= kl, np.float32(0), neg),
         np.zeros((128, 128), np.float32),
         np.where(ql < kl, np.float32(0), neg)], axis=1).astype(np.float32)
    onesr = np.ones((1, 128), dtype=np.float32)

    def chunk_w(w_cols):  # [D, FPC] -> [128, NDC*FPC]
        return w_cols.reshape(NDC, 128, FPC).transpose(1, 0, 2).reshape(
            128, NDC * FPC)

    in_maps = []
    for core in range(NCORES):
        b, hg = core // 4, core % 4
        cols = slice(hg * FPC, (hg + 1) * FPC)
        kcols = slice(D + hg * FPC, D + (hg + 1) * FPC)
        vcols = slice(2 * D + hg * FPC, 2 * D + (hg + 1) * FPC)
        rows = slice(hg * FPC, (hg + 1) * FPC)
        wqkv = np.concatenate(
            [chunk_w(w_attn[:, cols] * np.float32(SCALE)),
             chunk_w(w_attn[:, kcols]),
             chunk_w(w_attn[:, vcols])], axis=1)
        bqk = np.stack(
            [(b_attn[cols] * np.float32(SCALE)).reshape(2, 128)[0],
             (b_attn[cols] * np.float32(SCALE)).reshape(2, 128)[1],
             b_attn[kcols].reshape(2, 128)[0],
             b_attn[kcols].reshape(2, 128)[1]], axis=1)
        in_maps.append({
            "xb": x[b],
            "wqkv": np.ascontiguousarray(wqkv),
            "wp": np.ascontiguousarray(
                w_proj[rows, :].reshape(2, 128, D).transpose(1, 0, 2).reshape(
                    128, 2 * D)),
            "bqk": np.ascontiguousarray(bqk),
            "bv": b_attn[vcols].reshape(1, FPC).copy(),
            "ident_r": ident,
            "madd": madd,
            "onesr": onesr,
        })
    return in_maps


def kernel(x, w_attn, b_attn, w_proj, b_proj):
    if "nc" not in _CACHE:
        _CACHE["nc"] = _build_program()
    nc = _CACHE["nc"]

    in_maps = _prep_in_maps(x, w_attn, b_attn, w_proj)
    res = run_bass_kernel_spmd(nc, in_maps, core_ids=list(range(NCORES)))

    b_proj = np.asarray(b_proj, dtype=np.float32)
    out = np.zeros((B, S, D), dtype=np.float32)
    present = np.zeros((B, 2, N_HEAD, S, HD), dtype=np.float32)
    for core in range(NCORES):
        b, hg = core // 4, core % 4
        r = res.results[core]
        out[b] += r["outp"]
        present[b, 0, hg * HPC:(hg + 1) * HPC] = r["pk"]
        present[b, 1, hg * HPC:(hg + 1) * HPC] = r["pv"]
    out += b_proj
    return out, present


# revision 10
# speedup vs baseline: 1.4823x; 1.0407x over previous
"""Trainium2 Bass kernel for sliding-window causal attention block.

Reference computation (B=2, S=2048, D=1024, H=16, hd=64, WINDOW=256):
    c = x @ w_attn + b_attn ; q,k,v = split(c)
    present = stack([k, v]) as [B,2,H,S,hd]
    att = softmax(mask(q k^T / sqrt(hd))) @ v
    out = att @ w_proj + b_proj

Sharding: 8 cores = 2 batches x 4 head-groups (4 heads each).
Per core: QKV projection for its 256 q/k/v features (column-sharded
w_attn), attention for its 4 heads, and a partial out-projection
(row-sharded w_proj). Host sums the 4 partials per batch and adds
b_proj exactly.

Layout strategy on-core:
  x[b] is PE-transposed once to xT [D, S] (f32r, 4 transposes batched
  per PSUM bank, eviction split across DVE/ACT).
  Q^T, K^T produced feature-major [feat, S] (f32r), V natural [S, feat].
  Scores are computed directly transposed: S^T[kpos, q] tile per
  128-kpos chunk covering its 384 valid q columns (window 256 spans 3
  q-tiles), so softmax needs no P transposes. Masks: causal triangle on
  the left third, all-valid middle, anti-causal triangle right.
  exp on ACT -> P^T fp16; AV matmul uses V|ones fp16 where columns
  64:128 are all ones, so the softmax denominator lands replicated on
  PSUM partitions 64:128 -> 64-lane reciprocal + multiply on DVE.
  O^T f32r; out-projection back to natural [S, D] layout.
  DMAs are batched aggressively (the SP sequencer pays ~0.6us per DMA
  instruction): 8 input DMAs for x, 1 for all of wq/wk/wv, single
  per-head DMAs for the present k/v outputs, 2-S-tile DMAs for out.
"""

import sys

sys.path.insert(0, "/opt/trn_rl_repo")

import numpy as np

import concourse.bass as bass  # noqa: F401  (bass must import before bacc)
import concourse.mybir as mybir
from concourse import bacc
from concourse.tile import TileContext
from concourse.bass_utils import run_bass_kernel_spmd

F32 = mybir.dt.float32
F32R = mybir.dt.float32r
F16 = mybir.dt.float16

B, S, D = 2, 2048, 1024
N_HEAD = 16
HD = 64
WINDOW = 256
NCORES = 8
HPC = N_HEAD // 4  # heads per core = 4
FPC = HPC * HD  # features per core = 256
NQT = S // 128  # 16 q/kpos tiles
NDC = D // 128  # 8 contraction chunks
SCALE = 1.0 / np.sqrt(HD)

_CACHE = {}


def _build_program():
    nc = bacc.Bacc("TRN2", target_bir_lowering=False, debug=False,
                   num_devices=NCORES)

    # ---- DRAM I/O ----
    xb = nc.dram_tensor("xb", [S, D], F32R, kind="ExternalInput")
    # wq | wk | wv, each pre-chunked to [128, NDC*FPC]
    wqkv = nc.dram_tensor("wqkv", [128, 3 * NDC * FPC], F32R,
                          kind="ExternalInput")
    wp = nc.dram_tensor("wp", [128, 2 * D], F32R, kind="ExternalInput")
    bqk = nc.dram_tensor("bqk", [128, 4], F32, kind="ExternalInput")
    bv = nc.dram_tensor("bv", [1, FPC], F32R, kind="ExternalInput")
    ident_r = nc.dram_tensor("ident_r", [128, 128], F32R, kind="ExternalInput")
    madd = nc.dram_tensor("madd", [128, 384], F32R, kind="ExternalInput")
    onesr = nc.dram_tensor("onesr", [1, 128], F32R, kind="ExternalInput")

    outp = nc.dram_tensor("outp", [S, D], F32, kind="ExternalOutput")
    pk = nc.dram_tensor("pk", [HPC, S, HD], F32, kind="ExternalOutput")
    pv = nc.dram_tensor("pv", [HPC, S, HD], F32, kind="ExternalOutput")

    Exp = mybir.ActivationFunctionType.Exp
    Ident = mybir.ActivationFunctionType.Identity

    with TileContext(nc) as tc:
        with tc.tile_pool(name="const", bufs=1) as cpool, \
             tc.tile_pool(name="qkv", bufs=1) as qkv, \
             tc.tile_pool(name="ot", bufs=1) as otp:

            # identity first (x transposes depend on it)
            idr_sb = cpool.tile([128, 128], F32R, tag="idr")
            nc.sync.dma_start(out=idr_sb[:], in_=ident_r[:])

            # persistent activations
            qt_sb = [qkv.tile([128, S], F32R, tag=f"qt{i}", name=f"qt{i}")
                     for i in range(2)]
            kt_sb = [qkv.tile([128, S], F32R, tag=f"kt{i}", name=f"kt{i}")
                     for i in range(2)]
            v_sb = qkv.tile([128, NQT * FPC], F32, tag="v")
            # V|ones fp16 per head: head hl block [128, NQT*128];
            # cols 64:128 of each chunk stay 1.0 (denominator trick)
            vb_sb = qkv.tile([128, HPC * NQT * 128], F16, tag="vb")
            ot_sb = [otp.tile([128, S], F32R, tag=f"ot{i}", name=f"ot{i}")
                     for i in range(2)]

            nc.gpsimd.memset(vb_sb[:], 1.0)

            # ---- stage 1+2: x load/transpose + QKV projection ----
            with tc.tile_pool(name="xt", bufs=1) as xtp, \
                 tc.tile_pool(name="xload", bufs=2) as xlp, \
                 tc.tile_pool(name="tp_ps", bufs=2, space="PSUM") as tpps, \
                 tc.tile_pool(name="qk_ps", bufs=4, space="PSUM") as qkps, \
                 tc.tile_pool(name="vp_ps", bufs=2, space="PSUM") as vpps:

                xt_sb = xtp.tile([128, NDC * S], F32R, tag="xt")
                xt3 = xt_sb[:].rearrange("p (dc s) -> p dc s", dc=NDC)

                # weights first in emission (small, DMA queue drains while
                # x tiles stream)
                wqkv_sb = cpool.tile([128, 3 * NDC * FPC], F32R, tag="wqkv")
                nc.sync.dma_start(out=wqkv_sb[:], in_=wqkv[:])
                wq_sb = wqkv_sb[:, 0:NDC * FPC]
                wk_sb = wqkv_sb[:, NDC * FPC:2 * NDC * FPC]
                wv_sb = wqkv_sb[:, 2 * NDC * FPC:3 * NDC * FPC]
                bqk_sb = cpool.tile([128, 4], F32, tag="bqk")
                nc.sync.dma_start(out=bqk_sb[:], in_=bqk[:])
                bv_sb = cpool.tile([1, FPC], F32R, tag="bv")
                nc.sync.dma_start(out=bv_sb[:], in_=bv[:])
                madd_sb = cpool.tile([128, 384], F32R, tag="madd")
                nc.sync.dma_start(out=madd_sb[:], in_=madd[:])
                on_sb = cpool.tile([1, 128], F32R, tag="on")
                nc.sync.dma_start(out=on_sb[:], in_=onesr[:])

                # interleave x load/transpose with QKV so PE never waits
                # for the full x transfer (PE streams are in-order)
                for sc in range(4):
                    for st in range(4 * sc, 4 * sc + 4):
                        x_t = xlp.tile([128, D], F32R, tag="x",
                                       name=f"x{st}")
                        nc.sync.dma_start(
                            out=x_t[:],
                            in_=xb[st * 128:(st + 1) * 128, :])
                        for dg in range(2):
                            ps = tpps.tile([128, 512], F32R, tag="tp",
                                           name=f"tp{st}_{dg}")
                            for j in range(4):
                                dc = dg * 4 + j
                                nc.tensor.transpose(
                                    ps[:, j * 128:(j + 1) * 128],
                                    x_t[:, dc * 128:(dc + 1) * 128],
                                    idr_sb[:])
                            dst = xt3[:, dg * 4:(dg + 1) * 4,
                                      st * 128:st * 128 + 128]
                            srcp = ps[:].rearrange("p (j s) -> p j s", j=4)
                            if (st + dg) % 2 == 0:
                                nc.vector.tensor_copy(dst, srcp)
                            else:
                                nc.scalar.copy(dst, srcp)

                    # Q^T / K^T columns for this 512-wide S group.
                    # Q evicts on ACT (+bias), K evicts on DVE (+bias).
                    for wi, (w_sb, dstt) in enumerate(((wq_sb, qt_sb),
                                                       (wk_sb, kt_sb))):
                        for ft in range(2):
                            psq = qkps.tile([128, 512], F32, tag="qk",
                                            name=f"qkps{sc}_{wi}{ft}")
                            for dc in range(NDC):
                                lhsT = w_sb[:, dc * FPC + ft * 128:
                                            dc * FPC + ft * 128 + 128]
                                nc.tensor.matmul(
                                    psq[:], lhsT,
                                    xt_sb[:, dc * S + sc * 512:
                                          dc * S + sc * 512 + 512],
                                    start=(dc == 0), stop=(dc == NDC - 1))
                            bias_ap = bqk_sb[:, 2 * wi + ft: 2 * wi + ft + 1]
                            dslc = dstt[ft][:, sc * 512:(sc + 1) * 512]
                            if wi == 0:
                                nc.scalar.activation(dslc, psq[:], Ident,
                                                     bias=bias_ap)
                            else:
                                nc.vector.tensor_scalar_add(dslc, psq[:],
                                                            bias_ap)

                    # V natural + fused bias for these 4 S-tiles
                    for st in range(4 * sc, 4 * sc + 4):
                        vp = vpps.tile([128, FPC], F32, tag="vp",
                                       name=f"vp{st}")
                        for dc in range(NDC):
                            nc.tensor.matmul(
                                vp[:],
                                xt_sb[:, dc * S + st * 128:
                                      dc * S + st * 128 + 128],
                                wv_sb[:, dc * FPC:(dc + 1) * FPC],
                                start=(dc == 0), stop=False)
                        nc.tensor.matmul(vp[:], on_sb[:], bv_sb[:],
                                         start=False, stop=True)
                        nc.scalar.copy(v_sb[:, st * FPC:(st + 1) * FPC],
                                       vp[:])
                        srcv = vp[:].rearrange("p (hl c) -> p hl c", hl=HPC)
                        dst3 = vb_sb[:].rearrange(
                            "p (hl t) -> p hl t", hl=HPC)[
                            :, :, st * 128: st * 128 + HD]
                        if st % 2 == 0:
                            nc.vector.tensor_copy(dst3, srcv)
                        else:
                            nc.scalar.copy(dst3, srcv)

            # pools that reuse the space freed by xt: wp, K-natural
            # collector, attention transients, out staging
            with tc.tile_pool(name="late", bufs=1) as late, \
                 tc.tile_pool(name="pt", bufs=4) as ptp, \
                 tc.tile_pool(name="rs", bufs=4) as rsp, \
                 tc.tile_pool(name="osb", bufs=2) as osbp:

                wp_sb = late.tile([128, 2 * D], F32R, tag="wp")
                nc.sync.dma_start(out=wp_sb[:], in_=wp[:])
                kn_sb = late.tile([128, 2 * NQT * 128], F32, tag="kn")

                # ---- stage 3: K natural + present outputs ----
                with tc.tile_pool(name="kn_ps", bufs=3, space="PSUM") as knps:
                    for ft in range(2):
                        for st in range(NQT):
                            kp = knps.tile([128, 128], F32R, tag="knp")
                            nc.tensor.transpose(
                                kp[:], kt_sb[ft][:, st * 128:(st + 1) * 128],
                                idr_sb[:])
                            dstk = kn_sb[:, ft * S + st * 128:
                                         ft * S + st * 128 + 128]
                            if st % 2 == 0:
                                nc.scalar.copy(dstk, kp[:].bitcast(F32))
                            else:
                                nc.vector.tensor_copy(dstk, kp[:].bitcast(F32))
                    # one DMA per head
                    kn4 = kn_sb[:].rearrange(
                        "p (ft st h d) -> p ft st h d", ft=2, st=NQT, h=2)
                    for ft in range(2):
                        for h2 in range(2):
                            nc.sync.dma_start(
                                out=pk[2 * ft + h2, :, :].rearrange(
                                    "(st p) d -> p st d", p=128),
                                in_=kn4[:, ft, :, h2, :])
                    v4 = v_sb[:].rearrange(
                        "p (st hl d) -> p st hl d", st=NQT, hl=HPC)
                    for hl in range(HPC):
                        nc.sync.dma_start(
                            out=pv[hl, :, :].rearrange(
                                "(st p) d -> p st d", p=128),
                            in_=v4[:, :, hl, :])

                # ---- stage 4: attention per head (software-pipelined).
                # The last head also interleaves the out-projection so PE
                # fills exp-latency stalls with useful work. ----
                def attention_head(hl, stps, ops, norm_lag, per_chunk_hook):
                    ft, po = hl // 2, (hl % 2) * 64
                    kth = kt_sb[ft]
                    qth = qt_sb[ft]
                    oth = ot_sb[ft]
                    vbh = vb_sb[:, hl * NQT * 128:(hl + 1) * NQT * 128]

                    pts = [None] * NQT
                    osums = [None] * NQT

                    def qk_exp_mask(c):
                        qw = min(384, S - c * 128)
                        sp = stps.tile([128, 384], F32, tag="sp",
                                       name=f"sp{hl}_{c}")
                        nc.tensor.matmul(
                            sp[:, :qw],
                            kth[po:po + 64, c * 128:(c + 1) * 128],
                            qth[po:po + 64, c * 128:c * 128 + qw],
                            start=True, stop=False)
                        # additive mask (-1e30 off-window) via I.T @ madd
                        nc.tensor.matmul(
                            sp[:, :qw], idr_sb[:], madd_sb[:, :qw],
                            start=False, stop=True)
                        pt = ptp.tile([128, 384], F16, tag="pt",
                                      name=f"pt{hl}_{c}")
                        nc.scalar.activation(pt[:, :qw], sp[:, :qw], Exp)
                        pts[c] = pt

                    def osum(c):
                        if osums[c] is None:
                            osums[c] = ops.tile([128, 128], F32, tag="o",
                                                name=f"o{hl}_{c}")
                        return osums[c]

                    def normalize(c):
                        o_cur = osums[c]
                        rec = rsp.tile([64, 128], F32, tag="rec",
                                       name=f"rec{hl}_{c}")
                        nc.vector.reciprocal(rec[:], o_cur[64:128, :])
                        nc.vector.tensor_mul(
                            oth[po:po + 64, c * 128:(c + 1) * 128],
                            o_cur[0:64, :], rec[:])
                        osums[c] = None

                    qk_exp_mask(0)
                    qk_exp_mask(1)
                    for c in range(NQT):
                        if c + 2 < NQT:
                            qk_exp_mask(c + 2)
                        pt = pts[c]
                        qw = min(384, S - c * 128)
                        lhsT = vbh[:, c * 128:(c + 1) * 128]
                        nc.tensor.matmul(osum(c)[:], lhsT, pt[:, 0:128],
                                         start=(c == 0), stop=True)
                        if c + 1 < NQT:
                            nc.tensor.matmul(osum(c + 1)[:], lhsT,
                                             pt[:, 128:256],
                                             start=(c == 0), stop=False)
                        if c + 2 < NQT and qw > 256:
                            nc.tensor.matmul(osum(c + 2)[:], lhsT,
                                             pt[:, 256:384],
                                             start=True, stop=False)
                        # normalization lags the AV matmuls so the DVE
                        # stream never round-trips against PE
                        if c >= norm_lag:
                            normalize(c - norm_lag)
                            if per_chunk_hook is not None:
                                per_chunk_hook(c - norm_lag)
                        pts[c] = None
                    for c in range(NQT - norm_lag, NQT):
                        normalize(c)
                        if per_chunk_hook is not None:
                            per_chunk_hook(c)

                osb_state = {}

                def outproj_tile(st, opps):
                    # called once per q-tile st (in order) after all heads
                    # normalized it
                    st2, i = st // 2, st % 2
                    if i == 0:
                        osb_state["t"] = osbp.tile([128, 2 * D], F32,
                                                   tag="osb",
                                                   name=f"osb{st2}")
                    o_t = osb_state["t"]
                    for half in range(2):
                        op = opps.tile([128, 512], F32, tag="op",
                                       name=f"op{st}_{half}")
                        nc.tensor.matmul(
                            op[:], ot_sb[0][:, st * 128:(st + 1) * 128],
                            wp_sb[:, half * 512: half * 512 + 512],
                            start=True, stop=False)
                        nc.tensor.matmul(
                            op[:], ot_sb[1][:, st * 128:(st + 1) * 128],
                            wp_sb[:, D + half * 512: D + half * 512 + 512],
                            start=False, stop=True)
                        dsl = o_t[:, i * D + half * 512:
                                  i * D + (half + 1) * 512]
                        if half == 0:
                            nc.scalar.copy(dsl, op[:])
                        else:
                            nc.vector.tensor_copy(dsl, op[:])
                    if i == 1:
                        nc.sync.dma_start(
                            out=outp[st2 * 256:(st2 + 1) * 256, :].rearrange(
                                "(j p) d -> p j d", p=128),
                            in_=o_t[:].rearrange("p (j d) -> p j d", j=2))

                with tc.tile_pool(name="st_ps", bufs=3, space="PSUM") as stps, \
                     tc.tile_pool(name="o_ps", bufs=5, space="PSUM") as ops:
                    for hl in range(HPC - 1):
                        attention_head(hl, stps, ops, 2, None)

                with tc.tile_pool(name="st_ps2", bufs=2, space="PSUM") as stps, \
                     tc.tile_pool(name="o_ps2", bufs=4, space="PSUM") as ops, \
                     tc.tile_pool(name="op_ps", bufs=2, space="PSUM") as opps:
                    attention_head(HPC - 1, stps, ops, 1,
                                   lambda st: outproj_tile(st, opps))

    nc.compile()
    return nc


def _prep_in_maps(x, w_attn, b_attn, w_proj):
    """Per-core input dicts (host-side sharding + layout prep)."""
    x = np.ascontiguousarray(np.asarray(x, dtype=np.float32))
    w_attn = np.asarray(w_attn, dtype=np.float32)
    b_attn = np.asarray(b_attn, dtype=np.float32)
    w_proj = np.asarray(w_proj, dtype=np.float32)

    ident = np.eye(128, dtype=np.float32)
    ql = np.arange(128)[None, :]
    kl = np.arange(128)[:, None]
    neg = np.float32(-1e30)
    madd = np.concatenate(
        [np.where(ql >= kl, np.float32(0), neg),
         np.zeros((128, 128), np.float32),
         np.where(ql < kl, np.float32(0), neg)], axis=1).astype(np.float32)
    onesr = np.ones((1, 128), dtype=np.float32)

    def chunk_w(w_cols):  # [D, FPC] -> [128, NDC*FPC]
        return w_cols.reshape(NDC, 128, FPC).transpose(1, 0, 2).reshape(
            128, NDC * FPC)

    in_maps = []
    for core in range(NCORES):
        b, hg = core // 4, core % 4
        cols = slice(hg * FPC, (hg + 1) * FPC)
        kcols = slice(D + hg * FPC, D + (hg + 1) * FPC)
        vcols = slice(2 * D + hg * FPC, 2 * D + (hg + 1) * FPC)
        rows = slice(hg * FPC, (hg + 1) * FPC)
        wqkv = np.concatenate(
            [chunk_w(w_attn[:, cols] * np.float32(SCALE)),
             chunk_w(w_attn[:, kcols]),
             chunk_w(w_attn[:, vcols])], axis=1)
        bqk = np.stack(
            [(b_attn[cols] * np.float32(SCALE)).reshape(2, 128)[0],
             (b_attn[cols] * np.float32(SCALE)).reshape(2, 128)[1],
             b_attn[kcols].reshape(2, 128)[0],
             b_attn[kcols].reshape(2, 128)[1]], axis=1)
        in_maps.append({
            "xb": x[b],
            "wqkv": np.ascontiguousarray(wqkv),
            "wp": np.ascontiguousarray(
                w_proj[rows, :].reshape(2, 128, D).transpose(1, 0, 2).reshape(
                    128, 2 * D)),
            "bqk": np.ascontiguousarray(bqk),
            "bv": b_attn[vcols].reshape(1, FPC).copy(),
            "ident_r": ident,
            "madd": madd,
            "onesr": onesr,
        })
    return in_maps


def kernel(x, w_attn, b_attn, w_proj, b_proj):
    if "nc" not in _CACHE:
        _CACHE["nc"] = _build_program()
    nc = _CACHE["nc"]

    in_maps = _prep_in_maps(x, w_attn, b_attn, w_proj)
    res = run_bass_kernel_spmd(nc, in_maps, core_ids=list(range(NCORES)))

    b_proj = np.asarray(b_proj, dtype=np.float32)
    out = np.zeros((B, S, D), dtype=np.float32)
    present = np.zeros((B, 2, N_HEAD, S, HD), dtype=np.float32)
    for core in range(NCORES):
        b, hg = core // 4, core % 4
        r = res.results[core]
        out[b] += r["outp"]
        present[b, 0, hg * HPC:(hg + 1) * HPC] = r["pk"]
        present[b, 1, hg * HPC:(hg + 1) * HPC] = r["pv"]
    out += b_proj
    return out, present
